# revision 10
# baseline (speedup 1.0000x reference)
"""DGCNN (4x SAGEConv + SortPool + Conv1d + MLP) Trainium2 Bass kernel.

Sharding: data-parallel over the B=512 graphs -> 64 graphs per core on 8 cores.
Edges never cross graphs, so each core's message passing is local. The edge
list is converted on the host into a per-graph normalized adjacency
(AT[g][s,d] = multiplicity(s->d) / max(deg(d),1)); aggregation then becomes a
block-diagonal dense matmul on the PE array (2 graphs of 64 nodes per
128-partition tile).

All pre-sort math is exact fp32: the sort keys have adjacent gaps down to
~3e-7, so any lower-precision SAGE arithmetic flips ranks and destroys the
output. Post-sort values tolerate ~1e-3 noise, so the selection matmul runs
in fp16 and conv/lin1 in f32r.

SortPool is computed exactly (stable argsort semantics incl. ties, which are
common: ~59% of keys are exactly 0 post-relu) via a rank computation:
  rank(i) = #{j : k_j > k_i}  on keys perturbed by  k_i -= i*1e-11
(the perturbation resolves exact ties by index; distinct key values are never
closer than ~3e-7 on this data so the perturbation cannot reorder them).
Selection of the top-30 rows per graph is a one-hot matmul.

Conv1d is 4 accumulated [128,*]x[128,*] matmuls per output tile (im2col via
strided access patterns, never materialized). lin1's 7MB weight is prefetched
at tail start into SBUF freed by the SAGE layers (hT_b / h_sb scratch).
"""

import numpy as np

import concourse.bass as bass
import concourse.bacc as bacc
import concourse.mybir as mybir
import concourse.tile as tile
from concourse.bass_utils import run_bass_kernel_spmd

B, P, K, KS = 512, 64, 30, 4
N, E, F, H = B * P, 524288, 128, 256
L_OUT = K - KS + 1          # 27
N_CLASSES = 10
N_CORES = 8
GPC = B // N_CORES          # 64 graphs / core
NPC = GPC * P               # 4096 nodes / core
PAIRS = GPC // 2            # 32 pair-tiles (2 graphs of 64 nodes = 128 partitions)
NCHUNK = 512                # free-dim chunk for weight matmuls
F32 = mybir.dt.float32
F32R = mybir.dt.float32r
F16 = mybir.dt.float16
EPS_TIE = 1e-11

NLAYERS = 4
DUMP = False
USE_F32R = True
GCHUNK = 16                 # graphs per conv psum tile (16*28 = 448 <= 512)
L28 = L_OUT + 1             # conv free dim padded even (f32r ISA: innermost count even)
TKPAD = GPC * K + 8         # topkT free size incl. zeroed overrun pad
S1 = 2 * L_OUT              # 54 lin1 contraction steps of 128
W1A = 32                    # lin1 slabs prefetched into hT_b scratch
W1B = S1 - W1A              # lin1 slabs prefetched into h_sb scratch


# ---------------------------------------------------------------- host prep

def _prep_shared(inp):
    """Host-side weight/constant reshaping (identical for every core)."""
    sh = {}
    for li in range(4):
        sh[f"wl{li}"] = np.ascontiguousarray(inp[f"sage{li}_wl"], np.float32)
        sh[f"wr{li}"] = np.ascontiguousarray(inp[f"sage{li}_wr"], np.float32)
        sh[f"b{li}"] = np.ascontiguousarray(inp[f"sage{li}_b"], np.float32)
    w = np.asarray(inp["conv1d_w"], np.float32)            # [O=256, I=256, KS]
    w2 = np.empty((2 * KS, 128, H), np.float32)
    for k in range(KS):
        wt = w[:, :, k].T                                  # [I, O]
        for ih in range(2):
            w2[k * 2 + ih] = wt[ih * 128:(ih + 1) * 128]
    sh["w2"] = w2
    sh["cb"] = np.ascontiguousarray(inp["conv1d_b"], np.float32)
    w1 = np.asarray(inp["lin1_w"], np.float32)             # [6912, 256]
    sh["w1"] = np.ascontiguousarray(
        w1.reshape(2, 128, L_OUT, H).transpose(0, 2, 1, 3).reshape(S1, 128, H))
    sh["lb1"] = np.ascontiguousarray(
        np.broadcast_to(np.asarray(inp["lin1_b"], np.float32), (GPC, H)))
    sh["w4"] = np.ascontiguousarray(inp["lin2_w"], np.float32)   # [256, 128]
    sh["b2q"] = np.ascontiguousarray(inp["lin2_b"], np.float32)  # [128]
    sh["w5"] = np.ascontiguousarray(inp["out_w"], np.float32)    # [128, 10]
    sh["b3q"] = np.asarray(inp["out_b"], np.float32).reshape(N_CLASSES, 1).copy()
    sh["iota60"] = np.ascontiguousarray(
        np.broadcast_to(np.arange(2 * K, dtype=np.float32), (128, 2 * K)))
    off30 = np.zeros((128, 1), np.float32)
    off30[64:] = float(K)
    sh["off30"] = off30
    sh["epsrow"] = np.ascontiguousarray(
        np.broadcast_to(np.arange(P, dtype=np.float32) * np.float32(EPS_TIE), (P, P))).astype(np.float32)
    sh["id128"] = np.eye(128, dtype=np.float32)
    return sh


def _prep_cores(inp):
    """Per-core shards: node features (plain + transposed) and blockdiag adjacency."""
    x = np.nan_to_num(np.asarray(inp["x"], np.float32))
    ei = np.asarray(inp["edge_index"])
    src = ei[0].astype(np.int64)
    dst = ei[1].astype(np.int64)
    deg = np.bincount(dst, minlength=N).astype(np.float32)
    inv_deg = (1.0 / np.maximum(deg, 1.0)).astype(np.float32)
    g = src // P
    flat = g * (P * P) + (src % P) * P + (dst % P)
    AT = np.bincount(flat, minlength=B * P * P).astype(np.float32).reshape(B, P, P)
    AT *= inv_deg.reshape(B, P)[:, None, :]

    cores = []
    for c in range(N_CORES):
        xc = np.ascontiguousarray(x[c * NPC:(c + 1) * NPC])          # [4096, 128]
        atbd = np.zeros((PAIRS, 128, 128), np.float32)
        for t in range(PAIRS):
            atbd[t, :P, :P] = AT[c * GPC + 2 * t]
            atbd[t, P:, P:] = AT[c * GPC + 2 * t + 1]
        cores.append({
            "x": xc,
            "xt": np.ascontiguousarray(xc.T),                        # [128, 4096]
            "atbd": atbd,
        })
    return cores


# ---------------------------------------------------------------- device kernel

def _build(nc):
    """Emit the whole per-core kernel under a TileContext."""
    dt = nc.dram_tensor
    d_x = dt("x", [NPC, F], F32, kind="ExternalInput")
    WDT = F32R if USE_F32R else F32
    d_xt = dt("xt", [F, NPC], F32, kind="ExternalInput")
    d_atbd = dt("atbd", [PAIRS, 128, 128], F32, kind="ExternalInput")
    d_wl, d_wr, d_b = [], [], []
    for li in range(4):
        fin = F if li == 0 else H
        d_wl.append(dt(f"wl{li}", [fin, H], F32, kind="ExternalInput"))
        d_wr.append(dt(f"wr{li}", [fin, H], F32, kind="ExternalInput"))
        d_b.append(dt(f"b{li}", [H], F32, kind="ExternalInput"))
    d_w2 = dt("w2", [2 * KS, 128, H], WDT, kind="ExternalInput")
    d_cb = dt("cb", [H], F32, kind="ExternalInput")
    d_w1 = dt("w1", [S1, 128, H], F32, kind="ExternalInput")
    d_lb1 = dt("lb1", [GPC, H], F32, kind="ExternalInput")
    d_w4 = dt("w4", [H, 128], F32, kind="ExternalInput")
    d_b2q = dt("b2q", [128], F32, kind="ExternalInput")
    d_w5 = dt("w5", [128, N_CLASSES], F32, kind="ExternalInput")
    d_b3q = dt("b3q", [N_CLASSES, 1], F32, kind="ExternalInput")
    d_iota60 = dt("iota60", [128, 2 * K], F32, kind="ExternalInput")
    d_off30 = dt("off30", [128, 1], F32, kind="ExternalInput")
    d_epsrow = dt("epsrow", [P, P], F32, kind="ExternalInput")
    d_id128 = dt("id128", [128, 128], F32, kind="ExternalInput")
    d_out = dt("out", [GPC, N_CLASSES], F32, kind="ExternalOutput")
    if DUMP:
        d_dbg_ht = dt("dbg_ht", [128, 2, NPC], F32, kind="ExternalOutput")

    with tile.TileContext(nc) as tc:
        _emit(tc, nc, locals())
    nc.compile()
    return nc


def _ap(base, extra_offset, free_dims):
    """Build a custom AP view: keep base's partition dim, replace free dims."""
    return bass.AP(base.tensor, base.offset + extra_offset,
                   [base.ap[0]] + list(free_dims))


def _emit(tc, nc, d):
    WDT = F32R if USE_F32R else F32
    from contextlib import ExitStack
    ctx = ExitStack()
    with ctx:
        persist = ctx.enter_context(tc.tile_pool(name="persist", bufs=1))
        act_pool = ctx.enter_context(tc.tile_pool(name="acts", bufs=1))
        qs = [nc.sync, nc.scalar, nc.gpsimd]

        # ---- persistent loads (weight DMAs deferred until after input DMAs)
        _deferred = []

        def load(name, shape, view=None, dram=None, dtype=F32):
            t = persist.tile(shape, dtype, tag=name)
            src = (dram if dram is not None else d[f"d_{name}"]).ap()
            if view is not None:
                src = src.rearrange(*view[0], **view[1])
            _deferred.append((t, src))
            return t

        wl, wr, bias = [], [], []
        for li in range(4):
            ki = 1 if li == 0 else 2
            wl.append(load(f"wl{li}", [128, ki, H], (["(k p) o -> p k o"], {"p": 128}),
                           dram=d["d_wl"][li]))
            wr.append(load(f"wr{li}", [128, ki, H], (["(k p) o -> p k o"], {"p": 128}),
                           dram=d["d_wr"][li]))
            bias.append(load(f"b{li}", [128, 2], (["(h p) -> p h"], {"p": 128}),
                             dram=d["d_b"][li]))
            if li == 0:
                # needed by the first L0 transposes: load early
                id128 = load("id128", [128, 128])
        w2 = load("w2", [128, 2 * KS, H], (["k p o -> p k o"], {}), dtype=WDT)
        cb = load("cb", [128, 2], (["(h p) -> p h"], {"p": 128}))
        b1 = load("lb1", [GPC, H])
        w4 = load("w4", [128, 2, 128], (["(k p) o -> p k o"], {"p": 128}))
        b2q = load("b2q", [128, 1])
        w5 = load("w5", [128, N_CLASSES])
        b3q = load("b3q", [N_CLASSES, 1])
        iota60 = load("iota60", [128, 2 * K])
        off30 = load("off30", [128, 1])
        epsrow = load("epsrow", [P, P])

        # ---- activations (bufs=1: coarse WAR serialization at layer bounds is fine)
        # x is loaded directly into h_sb[:, :, 0:128] (L0 agg reads it there
        # before the L0 transposes overwrite it, tracked by tile deps).
        h_sb = act_pool.tile([128, PAIRS, H], F32, tag="h")       # nodes on partitions
        hT_a = act_pool.tile([128, 2, NPC], F32, tag="hTa")
        hT_b = act_pool.tile([128, 2, NPC], F32, tag="hTb")
        hts = [hT_a, hT_b]

        # ---- input x and aggregate (freed after the SAGE layers)
        with tc.tile_pool(name="xin", bufs=1) as xin:
            aggT_sb = xin.tile([128, 2, NPC], F32, tag="aggT")
            atbd_parts = []
            for g in range(4):
                src = d["d_x"].ap().rearrange("(t p) f -> p t f", p=128)[:, g * 8:(g + 1) * 8, :]
                qs[g % 3].dma_start(h_sb[:, g * 8:(g + 1) * 8, 0:F], src)
                t_at = persist.tile([128, PAIRS // 4, 128], F32, tag=f"atbd{g}",
                                    name=f"atbd{g}")
                srca = d["d_atbd"].ap().rearrange("t p n -> p t n")[:, g * 8:(g + 1) * 8, :]
                qs[(g + 1) % 3].dma_start(t_at[...], srca)
                atbd_parts.append(t_at)
                qs[(g + 2) % 3].dma_start(
                    hT_a[:, 0, g * 1024:(g + 1) * 1024],
                    d["d_xt"].ap()[:, g * 1024:(g + 1) * 1024])

            for _i, (_t, _src) in enumerate(_deferred):
                qs[_i % 3].dma_start(_t[...], _src)
            _deferred.clear()

            with tc.tile_pool(name="ps_sage", bufs=3, space="PSUM") as psa, \
                 tc.tile_pool(name="ps_w", bufs=2, space="PSUM") as psw, \
                 tc.tile_pool(name="ps_tr", bufs=2, space="PSUM") as pst:
                for li in range(NLAYERS):
                    ki = 1 if li == 0 else 2
                    hTv = hts[li % 2]                    # prev layer's hT (L0: xT half)
                    hTo = hts[(li + 1) % 2]              # this layer's output

                    # aggT[(i), n'] = h_pair^T @ ATbd_pair  per pair; 4 matmuls
                    # batched into one [128,512] psum tile -> wide copies
                    if li == 0:
                        for t4 in range(0, PAIRS, 4):
                            ps = psa.tile([128, 4 * 128], F32, tag="psa")
                            for j in range(4):
                                t = t4 + j
                                nc.tensor.matmul(
                                    ps[:, j * 128:(j + 1) * 128],
                                    lhsT=h_sb[:, t, 0:F],
                                    rhs=atbd_parts[t // 8][:, t % 8, :],
                                    start=True, stop=True)
                            nc.any.tensor_copy(
                                aggT_sb[:, 0, t4 * 128:(t4 + 4) * 128], ps[...])
                    else:
                        for t2 in range(0, PAIRS, 2):
                            ps = psa.tile([128, 4 * 128], F32, tag="psa")
                            col = 0
                            for mh in range(2):
                                for j in range(2):
                                    t = t2 + j
                                    nc.tensor.matmul(
                                        ps[:, col * 128:(col + 1) * 128],
                                        lhsT=h_sb[:, t, mh * 128:(mh + 1) * 128],
                                        rhs=atbd_parts[t // 8][:, t % 8, :],
                                        start=True, stop=True)
                                    col += 1
                            for mh in range(2):
                                nc.any.tensor_copy(
                                    aggT_sb[:, mh, t2 * 128:(t2 + 2) * 128],
                                    ps[:, mh * 256:(mh + 1) * 256])

                    # hT_next[o, n] = relu( wl^T aggT + wr^T hT + b )
                    # L3 computes oh=1 (the sort-key half) first so the sort can
                    # start while oh=0 is still on the PE.
                    for oh in ((1, 0) if li == NLAYERS - 1 else (0, 1)):
                        for ncki in range(NPC // NCHUNK):
                            sl = slice(ncki * NCHUNK, (ncki + 1) * NCHUNK)
                            ps = psw.tile([128, NCHUNK], F32, tag="psw")
                            step, nsteps = 0, 2 * ki
                            for wmat, rt in ((wl[li], aggT_sb), (wr[li], hTv)):
                                for kh in range(ki):
                                    rhs = rt[:, kh, sl]
                                    nc.tensor.matmul(
                                        ps[...],
                                        lhsT=wmat[:, kh, oh * 128:(oh + 1) * 128],
                                        rhs=rhs,
                                        start=(step == 0), stop=(step == nsteps - 1))
                                    step += 1
                            if oh == 0:
                                nc.scalar.activation(
                                    hTo[:, oh, sl], ps[...],
                                    mybir.ActivationFunctionType.Relu,
                                    bias=bias[li][:, oh:oh + 1])
                            else:
                                # relu on the vector engine: (ps + b) max 0
                                nc.vector.tensor_scalar(
                                    hTo[:, oh, sl], ps[...],
                                    bias[li][:, oh:oh + 1], 0.0,
                                    op0=mybir.AluOpType.add,
                                    op1=mybir.AluOpType.max)

                    # h_next = transpose(hT_next) per pair (PE transpose mode);
                    # both halves batched into one [128,256] psum -> one copy.
                    # L3's transposes happen in the tail (into fp16 h_sel).
                    if li < NLAYERS - 1:
                        for t in range(PAIRS):
                            ps = pst.tile([128, H], F32, tag="pst")
                            for oh in range(2):
                                nc.tensor.transpose(
                                    ps[:, oh * 128:(oh + 1) * 128],
                                    hTo[:, oh, t * 128:(t + 1) * 128],
                                    id128[...])
                            nc.any.tensor_copy(h_sb[:, t, :], ps[...])

        if DUMP:
            nc.sync.dma_start(d["d_dbg_ht"].ap(), hts[NLAYERS % 2][...])

        # ---------------- tail: sort + selection + conv + mlp.
        # hT_b and h_sb are dead after the L3 weight matmuls / aggs; their SBUF
        # is reused as the lin1 weight prefetch buffer (7MB streamed here,
        # hidden behind sort+selection+conv).
        with tc.tile_pool(name="sort", bufs=1) as sp, \
             tc.tile_pool(name="tail", bufs=1) as tp:
            w1qs = [nc.scalar, nc.gpsimd, nc.scalar, nc.gpsimd, nc.scalar, nc.gpsimd]
            for i in range(4):          # slabs 8i..8i+7 -> hT_b plane i//2
                s0 = 8 * i
                base = hT_b[:, i // 2, (i % 2) * 2048:(i % 2) * 2048 + 2048]
                dst = _ap(base, 0, [[H, 8], [1, H]])
                src = d["d_w1"].ap()[s0:s0 + 8].rearrange("s p h -> p s h")
                w1qs[i].dma_start(dst, src)
            for j in range(2):          # slabs 32+11j.. -> h_sb
                s0 = W1A + 11 * j
                dst = h_sb[:, 11 * j:11 * j + 11, :]
                src = d["d_w1"].ap()[s0:s0 + 11].rearrange("s p h -> p s h")
                w1qs[4 + j].dma_start(dst, src)

            hTo = hts[NLAYERS % 2]
            h_sel = tp.tile([128, PAIRS, H], F16, tag="hsel")
            with tc.tile_pool(name="ps_t3", bufs=2, space="PSUM") as pst3:
                for t in range(PAIRS):
                    ps = pst3.tile([128, H], F32, tag="pst3")
                    for oh in range(2):
                        nc.tensor.transpose(
                            ps[:, oh * 128:(oh + 1) * 128],
                            hTo[:, oh, t * 128:(t + 1) * 128],
                            id128[...])
                    nc.any.tensor_copy(h_sel[:, t, :], ps[...])

            # ---------------- sort: ranks of the last feature channel per graph
            rt = sp.tile([P, P], F32, tag="rt")
            with tc.tile_pool(name="sort_scratch", bufs=1) as ss:
                km = ss.tile([P, P], F32, tag="km")
                # keys: feature 255 = (hi=1, p=127); node n = g*64+i
                nc.sync.dma_start(km[...], hTo[127:128, 1, :])
                kmp = ss.tile([P, P], F32, tag="kmp")
                nc.vector.tensor_sub(kmp[...], km[...], epsrow[...])
                cbt = ss.tile([P, P * P], F32, tag="cbt")
                kb = kmp[:, :]
                in0 = _ap(kb, 0, [[0, P], kb.ap[1]])       # [g, i(bc), j]   k(g, j)
                in1 = _ap(kb, 0, [kb.ap[1], [0, P]])       # [g, i, j(bc)]   k(g, i)
                nc.vector.tensor_tensor(
                    _ap(cbt[:, :], 0, [[P, P], [1, P]]), in0, in1,
                    op=mybir.AluOpType.is_gt)
                rk = ss.tile([P, P], F32, tag="rk")
                nc.vector.tensor_reduce(
                    rk[...], _ap(cbt[:, :], 0, [[P, P], [1, P]]),
                    axis=mybir.AxisListType.X, op=mybir.AluOpType.add)
                # transpose ranks -> [node i, graph g]
                with tc.tile_pool(name="ps_sort", bufs=1, space="PSUM") as pss:
                    pr = pss.tile([P, P], F32, tag="pr")
                    nc.tensor.transpose(pr[...], rk[...], id128[0:P, 0:P])
                    nc.any.tensor_copy(rt[...], pr[...])
            # rankP[p, t] = rank(node p%64 of graph 2t + p//64)
            rankp = sp.tile([128, PAIRS], F32, tag="rankp")
            rb = rt[:, :]
            nc.vector.tensor_copy(rankp[0:P, :], _ap(rb, 0, [[2, PAIRS]]))
            nc.sync.dma_start(rankp[P:128, :], _ap(rb, 1, [[2, PAIRS]]))
            # rank2 = rankp + 30*(p>=64) + 1000*(rankp>=30)
            ge30 = sp.tile([128, PAIRS], F32, tag="ge30")
            nc.vector.tensor_scalar(ge30[...], rankp[...], float(K), None,
                                    op0=mybir.AluOpType.is_ge)
            rank2 = sp.tile([128, PAIRS], F32, tag="rank2")
            nc.vector.scalar_tensor_tensor(rank2[...], ge30[...], 1000.0,
                                           rankp[...], op0=mybir.AluOpType.mult,
                                           op1=mybir.AluOpType.add)
            nc.vector.tensor_scalar(rank2[...], rank2[...], off30[:, 0:1], None,
                                    op0=mybir.AluOpType.add)
            # one-hot selection matrices  PT[p, t, c] = (c == rank2[p, t])
            pt_all = sp.tile([128, PAIRS, 2 * K], F16, tag="pt")
            io = iota60[:, :]
            r2 = rank2[:, :]
            nc.vector.tensor_tensor(
                pt_all[...],
                _ap(io, 0, [[0, PAIRS], [1, 2 * K]]),
                _ap(r2, 0, [[1, PAIRS], [0, 2 * K]]),
                op=mybir.AluOpType.is_equal)

            # ---------------- selection + conv + mlp
            with tc.tile_pool(name="ps_tail", bufs=2, space="PSUM") as ptl, \
                 tc.tile_pool(name="ps_fin", bufs=1, space="PSUM") as pfin:
                # topkT[f, b*30+r] = sum_n h4[n, f] * PT[n, b(pair), r]
                # fp16 inputs (values only; ranks already decided) 2 pairs/psum
                topkT = tp.tile([128, 2, TKPAD], WDT, tag="topkT")
                nc.vector.memset(topkT[:, :, GPC * K:].bitcast(F32), 0.0)
                for t2 in range(0, PAIRS, 2):
                    ps = ptl.tile([128, 8 * K], F32, tag="pssel")
                    col = 0
                    for mh in range(2):
                        for j in range(2):
                            t = t2 + j
                            nc.tensor.matmul(
                                ps[:, col * 2 * K:(col + 1) * 2 * K],
                                lhsT=h_sel[:, t, mh * 128:(mh + 1) * 128],
                                rhs=pt_all[:, t, :],
                                start=True, stop=True)
                            col += 1
                    for mh in range(2):
                        nc.any.tensor_copy(
                            topkT[:, mh, t2 * 2 * K:(t2 + 2) * 2 * K],
                            ps[:, mh * 4 * K:(mh + 1) * 4 * K])

                # conv1d: y[p, oh, b, l] = relu(sum_{k, ih} w2^T topkT[:, b*30+l+k] + cb)
                y_sb = tp.tile([128, 2, GPC, L28], F32, tag="y")
                for oh in range(2):
                    for bc in range(GPC // GCHUNK):
                        ps = ptl.tile([128, GCHUNK, L28], F32, tag="psconv")
                        step = 0
                        for k in range(KS):
                            for ih in range(2):
                                base = topkT[:, ih, :]
                                rhs = _ap(base, bc * GCHUNK * K + k,
                                          [[K, GCHUNK], [1, L28]])
                                nc.tensor.matmul(
                                    ps[...],
                                    lhsT=w2[:, k * 2 + ih, oh * 128:(oh + 1) * 128],
                                    rhs=rhs,
                                    start=(step == 0), stop=(step == 2 * KS - 1))
                                step += 1
                        nc.scalar.activation(
                            y_sb[:, oh, bc * GCHUNK:(bc + 1) * GCHUNK, :], ps[...],
                            mybir.ActivationFunctionType.Relu,
                            bias=cb[:, oh:oh + 1])

                # lin1 (b-major): z1T[b, o] = relu(sum_s y_s^T @ w1_s + b1)
                # w1 slabs were prefetched into hT_b (0..31) and h_sb (32..53)
                ps1 = pfin.tile([GPC, H], F32, tag="ps1")
                for s in range(S1):
                    ot, l = divmod(s, L_OUT)
                    if s < W1A:
                        rhs = hT_b[:, s // 16, (s % 16) * H:(s % 16 + 1) * H]
                    else:
                        rhs = h_sb[:, s - W1A, :]
                    nc.tensor.matmul(
                        ps1[...],
                        lhsT=y_sb[:, ot, :, l],
                        rhs=rhs,
                        start=(s == 0), stop=(s == S1 - 1))
                z1t = tp.tile([GPC, H], F32, tag="z1t")
                nc.vector.tensor_add(z1t[...], ps1[...], b1[...])
                nc.scalar.activation(z1t[...], z1t[...],
                                     mybir.ActivationFunctionType.Relu, bias=0.0)
                # transpose z1T -> z1 [o on partitions]; all small psum tiles
                # below share one reused bank (sequential ops)
                z1 = tp.tile([128, 2, GPC], F32, tag="z1")
                for mh in range(2):
                    psz = pfin.tile([128, GPC], F32, tag="pfs")
                    nc.tensor.transpose(psz[...],
                                        z1t[:, mh * 128:(mh + 1) * 128],
                                        id128[0:GPC, 0:GPC])
                    nc.any.tensor_copy(z1[:, mh, :], psz[...])

                # lin2 + out
                ps2 = pfin.tile([128, GPC], F32, tag="pfs")
                for kh in range(2):
                    nc.tensor.matmul(ps2[...], lhsT=w4[:, kh, :], rhs=z1[:, kh, :],
                                     start=(kh == 0), stop=(kh == 1))
                z2 = tp.tile([128, GPC], F32, tag="z2")
                nc.scalar.activation(z2[...], ps2[...],
                                     mybir.ActivationFunctionType.Relu,
                                     bias=b2q[:, 0:1])
                ps3 = pfin.tile([128, GPC], F32, tag="pfs")
                nc.tensor.matmul(ps3[0:N_CLASSES, :], lhsT=w5[...], rhs=z2[...],
                                 start=True, stop=True)
                o_sb = tp.tile([N_CLASSES, GPC], F32, tag="osb")
                nc.scalar.activation(o_sb[...], ps3[0:N_CLASSES, :],
                                     mybir.ActivationFunctionType.Relu,
                                     bias=b3q[:, 0:1])
                nc.sync.dma_start(d["d_out"].ap().rearrange("b o -> o b"), o_sb[...])


# ---------------------------------------------------------------- entry point

_CACHED = {}


def _get_nc():
    if "nc" not in _CACHED:
        nc = bacc.Bacc("TRN2", target_bir_lowering=False, debug=False,
                       enable_asserts=True)
        _CACHED["nc"] = _build(nc)
    return _CACHED["nc"]


def make_in_maps(inputs):
    sh = _prep_shared(inputs)
    cores = _prep_cores(inputs)
    return [{**sh, **c} for c in cores]


TRACE = False


def kernel(**inputs):
    in_maps = make_in_maps(inputs)
    nc = _get_nc()
    res = run_bass_kernel_spmd(nc, in_maps, core_ids=list(range(N_CORES)),
                               trace=TRACE)
    _CACHED["last_res"] = res
    return np.concatenate([r["out"] for r in res.results], axis=0)


if __name__ == "__main__":
    import reference
    inputs = {k: np.asarray(v) for k, v in reference.setup_inputs().items()}
    out = kernel(**inputs)
    print("out", out.shape, out.dtype)


# revision 13
# speedup vs baseline: 1.1294x; 1.1294x over previous
"""DGCNN (4x SAGEConv + SortPool + Conv1d + MLP) Trainium2 Bass kernel.

Sharding: data-parallel over the B=512 graphs -> 64 graphs per core on 8 cores.
Edges never cross graphs, so each core's message passing is local. The edge
list is converted on the host into a per-graph normalized adjacency
(AT[g][s,d] = multiplicity(s->d) / max(deg(d),1)); aggregation then becomes a
block-diagonal dense matmul on the PE array (2 graphs of 64 nodes per
128-partition tile).

All pre-sort math is exact fp32: the sort keys have adjacent gaps down to
~3e-7, so any lower-precision SAGE arithmetic flips ranks and destroys the
output. Post-sort values tolerate ~1e-3 noise, so the selection matmul runs
in fp16 and conv/lin1 in f32r.

SortPool is computed exactly (stable argsort semantics incl. ties, which are
common: ~59% of keys are exactly 0 post-relu) via a rank computation:
  rank(i) = #{j : k_j > k_i}  on keys perturbed by  k_i -= i*1e-11
(the perturbation resolves exact ties by index; distinct key values are never
closer than ~3e-7 on this data so the perturbation cannot reorder them).
Selection of the top-30 rows per graph is a one-hot matmul.

Conv1d is 4 accumulated [128,*]x[128,*] matmuls per output tile (im2col via
strided access patterns, never materialized). lin1's 7MB weight is prefetched
at tail start into SBUF freed by the SAGE layers.
"""

import numpy as np

import concourse.bass as bass
import concourse.bacc as bacc
import concourse.mybir as mybir
import concourse.tile as tile
from concourse.bass_utils import run_bass_kernel_spmd

B, P, K, KS = 512, 64, 30, 4
N, E, F, H = B * P, 524288, 128, 256
L_OUT = K - KS + 1          # 27
N_CLASSES = 10
N_CORES = 8
GPC = B // N_CORES          # 64 graphs / core
NPC = GPC * P               # 4096 nodes / core
PAIRS = GPC // 2            # 32 pair-tiles (2 graphs of 64 nodes = 128 partitions)
NCHUNK = 512                # free-dim chunk for weight matmuls
F32 = mybir.dt.float32
F32R = mybir.dt.float32r
F16 = mybir.dt.float16
EPS_TIE = 1e-11

NLAYERS = 4
DUMP = False
USE_F32R = True
GCHUNK = 16                 # graphs per conv psum tile (16*28 = 448 <= 512)
L28 = L_OUT + 1             # conv free dim padded even (f32r ISA: innermost count even)
TKPAD = GPC * K + 8         # topkT free size incl. zeroed overrun pad
S1 = 2 * L_OUT              # 54 lin1 contraction steps of 128
W1A = 32                    # lin1 slabs prefetched into a fresh tail tile
W1B = S1 - W1A              # lin1 slabs prefetched into h_sb scratch


# ---------------------------------------------------------------- host prep

def _sp16(a):
    """Split fp32 -> (hi, lo) fp16 pair with hi + lo ~= a to ~2^-22 rel."""
    a = np.asarray(a, np.float32)
    hi = a.astype(np.float16)
    lo = (a - hi.astype(np.float32)).astype(np.float16)
    return np.ascontiguousarray(hi), np.ascontiguousarray(lo)


def _prep_shared(inp):
    """Host-side weight/constant reshaping (identical for every core)."""
    sh = {}
    for li in range(4):
        for nm in ("wl", "wr"):
            hi, lo = _sp16(inp[f"sage{li}_{nm}"])
            sh[f"{nm}{li}h"] = hi
            sh[f"{nm}{li}l"] = lo
        sh[f"b{li}"] = np.ascontiguousarray(inp[f"sage{li}_b"], np.float32)
    w = np.asarray(inp["conv1d_w"], np.float32)            # [O=256, I=256, KS]
    w2 = np.empty((2 * KS, 128, H), np.float32)
    for k in range(KS):
        wt = w[:, :, k].T                                  # [I, O]
        for ih in range(2):
            w2[k * 2 + ih] = wt[ih * 128:(ih + 1) * 128]
    sh["w2"] = w2
    sh["cb"] = np.ascontiguousarray(inp["conv1d_b"], np.float32)
    w1 = np.asarray(inp["lin1_w"], np.float32)             # [6912, 256]
    sh["w1"] = np.ascontiguousarray(
        w1.reshape(2, 128, L_OUT, H).transpose(0, 2, 1, 3).reshape(S1, 128, H))
    sh["lb1"] = np.ascontiguousarray(
        np.broadcast_to(np.asarray(inp["lin1_b"], np.float32), (GPC, H)))
    sh["w4"] = np.ascontiguousarray(inp["lin2_w"], np.float32)   # [256, 128]
    sh["b2q"] = np.ascontiguousarray(inp["lin2_b"], np.float32)  # [128]
    sh["w5"] = np.ascontiguousarray(inp["out_w"], np.float32)    # [128, 10]
    sh["b3q"] = np.asarray(inp["out_b"], np.float32).reshape(N_CLASSES, 1).copy()
    sh["iota60"] = np.ascontiguousarray(
        np.broadcast_to(np.arange(2 * K, dtype=np.float32), (128, 2 * K)))
    off30 = np.zeros((128, 1), np.float32)
    off30[64:] = float(K)
    sh["off30"] = off30
    sh["epsrow"] = np.ascontiguousarray(
        np.broadcast_to(np.arange(P, dtype=np.float32) * np.float32(EPS_TIE), (P, P))).astype(np.float32)
    sh["id128"] = np.eye(128, dtype=np.float32)
    return sh


def _prep_cores(inp):
    """Per-core shards: node features (plain + transposed) and blockdiag adjacency."""
    x = np.nan_to_num(np.asarray(inp["x"], np.float32))
    ei = np.asarray(inp["edge_index"])
    src = ei[0].astype(np.int64)
    dst = ei[1].astype(np.int64)
    deg = np.bincount(dst, minlength=N).astype(np.float32)
    inv_deg = (1.0 / np.maximum(deg, 1.0)).astype(np.float32)
    g = src // P
    flat = g * (P * P) + (src % P) * P + (dst % P)
    AT = np.bincount(flat, minlength=B * P * P).astype(np.float32).reshape(B, P, P)
    AT *= inv_deg.reshape(B, P)[:, None, :]

    cores = []
    for c in range(N_CORES):
        xc = np.ascontiguousarray(x[c * NPC:(c + 1) * NPC])          # [4096, 128]
        atbd = np.zeros((PAIRS, 128, 128), np.float32)
        for t in range(PAIRS):
            atbd[t, :P, :P] = AT[c * GPC + 2 * t]
            atbd[t, P:, P:] = AT[c * GPC + 2 * t + 1]
        xth, xtl = _sp16(xc.T)                                   # [128, 4096]
        cores.append({
            "x": xc,
            "xth": xth,
            "xtl": xtl,
            "atbd": atbd,
        })
    return cores


# ---------------------------------------------------------------- device kernel

def _build(nc):
    """Emit the whole per-core kernel under a TileContext."""
    dt = nc.dram_tensor
    d_x = dt("x", [NPC, F], F32, kind="ExternalInput")
    WDT = F32R if USE_F32R else F32
    d_xth = dt("xth", [F, NPC], F16, kind="ExternalInput")
    d_xtl = dt("xtl", [F, NPC], F16, kind="ExternalInput")
    d_atbd = dt("atbd", [PAIRS, 128, 128], F32, kind="ExternalInput")
    d_wlh, d_wll, d_wrh, d_wrl, d_b = [], [], [], [], []
    for li in range(4):
        fin = F if li == 0 else H
        d_wlh.append(dt(f"wl{li}h", [fin, H], F16, kind="ExternalInput"))
        d_wll.append(dt(f"wl{li}l", [fin, H], F16, kind="ExternalInput"))
        d_wrh.append(dt(f"wr{li}h", [fin, H], F16, kind="ExternalInput"))
        d_wrl.append(dt(f"wr{li}l", [fin, H], F16, kind="ExternalInput"))
        d_b.append(dt(f"b{li}", [H], F32, kind="ExternalInput"))
    d_w2 = dt("w2", [2 * KS, 128, H], WDT, kind="ExternalInput")
    d_cb = dt("cb", [H], F32, kind="ExternalInput")
    d_w1 = dt("w1", [S1, 128, H], F32, kind="ExternalInput")
    d_lb1 = dt("lb1", [GPC, H], F32, kind="ExternalInput")
    d_w4 = dt("w4", [H, 128], F32, kind="ExternalInput")
    d_b2q = dt("b2q", [128], F32, kind="ExternalInput")
    d_w5 = dt("w5", [128, N_CLASSES], F32, kind="ExternalInput")
    d_b3q = dt("b3q", [N_CLASSES, 1], F32, kind="ExternalInput")
    d_iota60 = dt("iota60", [128, 2 * K], F32, kind="ExternalInput")
    d_off30 = dt("off30", [128, 1], F32, kind="ExternalInput")
    d_epsrow = dt("epsrow", [P, P], F32, kind="ExternalInput")
    d_id128 = dt("id128", [128, 128], F32, kind="ExternalInput")
    d_out = dt("out", [GPC, N_CLASSES], F32, kind="ExternalOutput")
    if DUMP:
        d_dbg_ht = dt("dbg_ht", [128, 2, NPC], F32, kind="ExternalOutput")

    with tile.TileContext(nc) as tc:
        _emit(tc, nc, locals())
    nc.compile()
    return nc


def _ap(base, extra_offset, free_dims):
    """Build a custom AP view: keep base's partition dim, replace free dims."""
    return bass.AP(base.tensor, base.offset + extra_offset,
                   [base.ap[0]] + list(free_dims))


def _emit(tc, nc, d):
    WDT = F32R if USE_F32R else F32
    from contextlib import ExitStack
    ctx = ExitStack()
    with ctx:
        persist = ctx.enter_context(tc.tile_pool(name="persist", bufs=1))
        act_pool = ctx.enter_context(tc.tile_pool(name="acts", bufs=1))
        qs = [nc.sync, nc.scalar, nc.gpsimd]

        # ---- persistent loads (weight DMAs deferred until after input DMAs)
        _deferred = []

        def load(name, shape, view=None, dram=None, dtype=F32):
            t = persist.tile(shape, dtype, tag=name)
            src = (dram if dram is not None else d[f"d_{name}"]).ap()
            if view is not None:
                src = src.rearrange(*view[0], **view[1])
            _deferred.append((t, src))
            return t

        wlh, wll, wrh, wrl, bias = [], [], [], [], []
        for li in range(4):
            ki = 1 if li == 0 else 2
            vw = (["(k p) o -> p k o"], {"p": 128})
            wlh.append(load(f"wl{li}h", [128, ki, H], vw, dram=d["d_wlh"][li], dtype=F16))
            wll.append(load(f"wl{li}l", [128, ki, H], vw, dram=d["d_wll"][li], dtype=F16))
            wrh.append(load(f"wr{li}h", [128, ki, H], vw, dram=d["d_wrh"][li], dtype=F16))
            wrl.append(load(f"wr{li}l", [128, ki, H], vw, dram=d["d_wrl"][li], dtype=F16))
            bias.append(load(f"b{li}", [128, 2], (["(h p) -> p h"], {"p": 128}),
                             dram=d["d_b"][li]))
            if li == 0:
                # needed by the first L0 transposes: load early
                id128 = load("id128", [128, 128])
        w2 = load("w2", [128, 2 * KS, H], (["k p o -> p k o"], {}), dtype=WDT)
        cb = load("cb", [128, 2], (["(h p) -> p h"], {"p": 128}))
        b1 = load("lb1", [GPC, H])
        w4 = load("w4", [128, 2, 128], (["(k p) o -> p k o"], {"p": 128}))
        b2q = load("b2q", [128, 1])
        w5 = load("w5", [128, N_CLASSES])
        b3q = load("b3q", [N_CLASSES, 1])
        iota60 = load("iota60", [128, 2 * K])
        off30 = load("off30", [128, 1])
        epsrow = load("epsrow", [P, P])

        # ---- activations (bufs=1: coarse WAR serialization at layer bounds is fine)
        # x is loaded directly into h_sb[:, :, 0:128] (L0 agg reads it there
        # before the L0 transposes overwrite it, tracked by tile deps).
        h_sb = act_pool.tile([128, PAIRS, H], F32, tag="h")       # nodes on partitions
        # single fp32 hT: consumed only by this layer's transposes (strictly
        # before the next layer's relu rewrites it); the weight-matmul rhs is
        # the fp16 hi/lo split pair below.
        hT = act_pool.tile([128, 2, NPC], F32, tag="hT")

        # ---- input x and aggregate (freed after the SAGE layers)
        with tc.tile_pool(name="xin", bufs=1) as xin:
            aggTh = xin.tile([128, 2, NPC], F16, tag="aggTh")
            aggTl = xin.tile([128, 2, NPC], F16, tag="aggTl")
            xth_sb = xin.tile([128, 1, NPC], F16, tag="xth")
            xtl_sb = xin.tile([128, 1, NPC], F16, tag="xtl")
            # fp16 hi/lo split of the layer's hT (weight-matmul rhs). Single
            # buffered: layer li+1 overwrites column sl only after both of its
            # oh-chunks for sl consumed it (ncki-outer loop order below).
            hsph = xin.tile([128, 2, NPC], F16, tag="hsph")
            hspl = xin.tile([128, 2, NPC], F16, tag="hspl")
            atbd_parts = []
            for g in range(4):
                src = d["d_x"].ap().rearrange("(t p) f -> p t f", p=128)[:, g * 8:(g + 1) * 8, :]
                qs[g % 3].dma_start(h_sb[:, g * 8:(g + 1) * 8, 0:F], src)
                t_at = persist.tile([128, PAIRS // 4, 128], F32, tag=f"atbd{g}",
                                    name=f"atbd{g}")
                srca = d["d_atbd"].ap().rearrange("t p n -> p t n")[:, g * 8:(g + 1) * 8, :]
                qs[(g + 1) % 3].dma_start(t_at[...], srca)
                atbd_parts.append(t_at)
                qs[(g + 2) % 3].dma_start(
                    xth_sb[:, 0, g * 1024:(g + 1) * 1024],
                    d["d_xth"].ap()[:, g * 1024:(g + 1) * 1024])
                qs[g % 3].dma_start(
                    xtl_sb[:, 0, g * 1024:(g + 1) * 1024],
                    d["d_xtl"].ap()[:, g * 1024:(g + 1) * 1024])

            for _i, (_t, _src) in enumerate(_deferred):
                qs[_i % 3].dma_start(_t[...], _src)
            _deferred.clear()

            with tc.tile_pool(name="ps_sage", bufs=3, space="PSUM") as psa, \
                 tc.tile_pool(name="ps_w", bufs=2, space="PSUM") as psw, \
                 tc.tile_pool(name="ps_tr", bufs=2, space="PSUM") as pst:
                for li in range(NLAYERS):
                    ki = 1 if li == 0 else 2
                    hTo = hT

                    # aggT[(i), n'] = h_pair^T @ ATbd_pair  per pair; 4 matmuls
                    # batched into one [128,512] psum tile -> wide copies
                    if li == 0:
                        for t4 in range(0, PAIRS, 4):
                            ps = psa.tile([128, 4 * 128], F32, tag="psa")
                            for j in range(4):
                                t = t4 + j
                                nc.tensor.matmul(
                                    ps[:, j * 128:(j + 1) * 128],
                                    lhsT=h_sb[:, t, 0:F],
                                    rhs=atbd_parts[t // 8][:, t % 8, :],
                                    start=True, stop=True)
                            dh = aggTh[:, 0, t4 * 128:(t4 + 4) * 128]
                            nc.any.tensor_copy(dh, ps[...])
                            nc.any.tensor_tensor(
                                aggTl[:, 0, t4 * 128:(t4 + 4) * 128],
                                ps[...], dh, op=mybir.AluOpType.subtract)
                    else:
                        for t2 in range(0, PAIRS, 2):
                            ps = psa.tile([128, 4 * 128], F32, tag="psa")
                            col = 0
                            for mh in range(2):
                                for j in range(2):
                                    t = t2 + j
                                    nc.tensor.matmul(
                                        ps[:, col * 128:(col + 1) * 128],
                                        lhsT=h_sb[:, t, mh * 128:(mh + 1) * 128],
                                        rhs=atbd_parts[t // 8][:, t % 8, :],
                                        start=True, stop=True)
                                    col += 1
                            for mh in range(2):
                                dh = aggTh[:, mh, t2 * 128:(t2 + 2) * 128]
                                nc.any.tensor_copy(dh, ps[:, mh * 256:(mh + 1) * 256])
                                nc.any.tensor_tensor(
                                    aggTl[:, mh, t2 * 128:(t2 + 2) * 128],
                                    ps[:, mh * 256:(mh + 1) * 256], dh,
                                    op=mybir.AluOpType.subtract)

                    # hT_next[o, n] = relu( wl^T aggT + wr^T hT + b )
                    # L3 computes oh=1 (the sort-key half) first so the sort can
                    # start while oh=0 is still on the PE.
                    if li == 0:
                        hh, hl = xth_sb, xtl_sb
                    else:
                        hh, hl = hsph, hspl

                    def _wchunk(oh, ncki):
                        sl = slice(ncki * NCHUNK, (ncki + 1) * NCHUNK)
                        ps = psw.tile([128, NCHUNK], F32, tag="psw", name="psw")
                        step, nsteps = 0, 2 * 3 * ki
                        for whi, wlo_, rth, rtl in ((wlh[li], wll[li], aggTh, aggTl),
                                                    (wrh[li], wrl[li], hh, hl)):
                            for kh in range(ki):
                                for wm, rt in ((whi, rth), (whi, rtl), (wlo_, rth)):
                                    nc.tensor.matmul(
                                        ps[...],
                                        lhsT=wm[:, kh, oh * 128:(oh + 1) * 128],
                                        rhs=rt[:, kh, sl],
                                        start=(step == 0), stop=(step == nsteps - 1))
                                    step += 1
                        if oh == 0:
                            nc.scalar.activation(
                                hTo[:, oh, sl], ps[...],
                                mybir.ActivationFunctionType.Relu,
                                bias=bias[li][:, oh:oh + 1])
                        else:
                            # relu on the vector engine: (ps + b) max 0
                            nc.vector.tensor_scalar(
                                hTo[:, oh, sl], ps[...],
                                bias[li][:, oh:oh + 1], 0.0,
                                op0=mybir.AluOpType.add,
                                op1=mybir.AluOpType.max)

                    if li == NLAYERS - 1:
                        # keys (oh=1 plane) first so the sort starts early
                        for oh in (1, 0):
                            for ncki in range(NPC // NCHUNK):
                                _wchunk(oh, ncki)
                    else:
                        for ncki in range(NPC // NCHUNK):
                            for oh in (0, 1):
                                _wchunk(oh, ncki)
                            # split columns sl for the next layer's fp16x2
                            # matmuls: hi = fp16(h), lo = fp16(h - hi)
                            sl = slice(ncki * NCHUNK, (ncki + 1) * NCHUNK)
                            for oh in (0, 1):
                                nc.any.tensor_copy(hsph[:, oh, sl], hTo[:, oh, sl])
                                nc.any.tensor_tensor(
                                    hspl[:, oh, sl], hTo[:, oh, sl],
                                    hsph[:, oh, sl], op=mybir.AluOpType.subtract)

                    # h_next = transpose(hT_next) per pair (PE transpose mode);
                    # both halves batched into one [128,256] psum -> one copy.
                    # L3's transposes happen in the tail (into fp16 h_sel).
                    if li < NLAYERS - 1:
                        for t in range(PAIRS):
                            ps = pst.tile([128, H], F32, tag="pst")
                            for oh in range(2):
                                nc.tensor.transpose(
                                    ps[:, oh * 128:(oh + 1) * 128],
                                    hTo[:, oh, t * 128:(t + 1) * 128],
                                    id128[...])
                            nc.any.tensor_copy(h_sb[:, t, :], ps[...])

        if DUMP:
            nc.sync.dma_start(d["d_dbg_ht"].ap(), hT[...])

        # ---------------- tail: sort + selection + conv + mlp.
        # hT_b and h_sb are dead after the L3 weight matmuls / aggs; their SBUF
        # is reused as the lin1 weight prefetch buffer (7MB streamed here,
        # hidden behind sort+selection+conv).
        with tc.tile_pool(name="sort", bufs=1) as sp, \
             tc.tile_pool(name="tail", bufs=1) as tp:
            w1qs = [nc.scalar, nc.gpsimd, nc.scalar, nc.gpsimd, nc.scalar, nc.gpsimd]
            w1a = tp.tile([128, W1A, H], F32, tag="w1a")
            for i in range(4):          # slabs 8i..8i+7 -> w1a
                s0 = 8 * i
                dst = w1a[:, s0:s0 + 8, :]
                src = d["d_w1"].ap()[s0:s0 + 8].rearrange("s p h -> p s h")
                w1qs[i].dma_start(dst, src)
            for j in range(2):          # slabs 32+11j.. -> h_sb
                s0 = W1A + 11 * j
                dst = h_sb[:, 11 * j:11 * j + 11, :]
                src = d["d_w1"].ap()[s0:s0 + 11].rearrange("s p h -> p s h")
                w1qs[4 + j].dma_start(dst, src)

            hTo = hT
            h_sel = tp.tile([128, PAIRS, H], F16, tag="hsel")
            with tc.tile_pool(name="ps_t3", bufs=2, space="PSUM") as pst3:
                for t in range(PAIRS):
                    ps = pst3.tile([128, H], F32, tag="pst3")
                    for oh in range(2):
                        nc.tensor.transpose(
                            ps[:, oh * 128:(oh + 1) * 128],
                            hTo[:, oh, t * 128:(t + 1) * 128],
                            id128[...])
                    nc.any.tensor_copy(h_sel[:, t, :], ps[...])

            # ---------------- sort: ranks of the last feature channel per graph
            rt = sp.tile([P, P], F32, tag="rt")
            with tc.tile_pool(name="sort_scratch", bufs=1) as ss:
                km = ss.tile([P, P], F32, tag="km")
                # keys: feature 255 = (hi=1, p=127); node n = g*64+i
                nc.sync.dma_start(km[...], hTo[127:128, 1, :])
                kmp = ss.tile([P, P], F32, tag="kmp")
                nc.vector.tensor_sub(kmp[...], km[...], epsrow[...])
                cbt = ss.tile([P, P * P], F32, tag="cbt")
                kb = kmp[:, :]
                in0 = _ap(kb, 0, [[0, P], kb.ap[1]])       # [g, i(bc), j]   k(g, j)
                in1 = _ap(kb, 0, [kb.ap[1], [0, P]])       # [g, i, j(bc)]   k(g, i)
                nc.vector.tensor_tensor(
                    _ap(cbt[:, :], 0, [[P, P], [1, P]]), in0, in1,
                    op=mybir.AluOpType.is_gt)
                rk = ss.tile([P, P], F32, tag="rk")
                nc.vector.tensor_reduce(
                    rk[...], _ap(cbt[:, :], 0, [[P, P], [1, P]]),
                    axis=mybir.AxisListType.X, op=mybir.AluOpType.add)
                # transpose ranks -> [node i, graph g]
                with tc.tile_pool(name="ps_sort", bufs=1, space="PSUM") as pss:
                    pr = pss.tile([P, P], F32, tag="pr")
                    nc.tensor.transpose(pr[...], rk[...], id128[0:P, 0:P])
                    nc.any.tensor_copy(rt[...], pr[...])
            # rankP[p, t] = rank(node p%64 of graph 2t + p//64)
            rankp = sp.tile([128, PAIRS], F32, tag="rankp")
            rb = rt[:, :]
            nc.vector.tensor_copy(rankp[0:P, :], _ap(rb, 0, [[2, PAIRS]]))
            nc.sync.dma_start(rankp[P:128, :], _ap(rb, 1, [[2, PAIRS]]))
            # rank2 = rankp + 30*(p>=64) + 1000*(rankp>=30)
            ge30 = sp.tile([128, PAIRS], F32, tag="ge30")
            nc.vector.tensor_scalar(ge30[...], rankp[...], float(K), None,
                                    op0=mybir.AluOpType.is_ge)
            rank2 = sp.tile([128, PAIRS], F32, tag="rank2")
            nc.vector.scalar_tensor_tensor(rank2[...], ge30[...], 1000.0,
                                           rankp[...], op0=mybir.AluOpType.mult,
                                           op1=mybir.AluOpType.add)
            nc.vector.tensor_scalar(rank2[...], rank2[...], off30[:, 0:1], None,
                                    op0=mybir.AluOpType.add)
            # one-hot selection matrices  PT[p, t, c] = (c == rank2[p, t])
            pt_all = sp.tile([128, PAIRS, 2 * K], F16, tag="pt")
            io = iota60[:, :]
            r2 = rank2[:, :]
            nc.vector.tensor_tensor(
                pt_all[...],
                _ap(io, 0, [[0, PAIRS], [1, 2 * K]]),
                _ap(r2, 0, [[1, PAIRS], [0, 2 * K]]),
                op=mybir.AluOpType.is_equal)

            # ---------------- selection + conv + mlp
            with tc.tile_pool(name="ps_tail", bufs=2, space="PSUM") as ptl, \
                 tc.tile_pool(name="ps_fin", bufs=1, space="PSUM") as pfin:
                # topkT[f, b*30+r] = sum_n h4[n, f] * PT[n, b(pair), r]
                # fp16 inputs (values only; ranks already decided) 2 pairs/psum
                topkT = tp.tile([128, 2, TKPAD], WDT, tag="topkT")
                nc.vector.memset(topkT[:, :, GPC * K:].bitcast(F32), 0.0)
                for t2 in range(0, PAIRS, 2):
                    ps = ptl.tile([128, 8 * K], F32, tag="pssel")
                    col = 0
                    for mh in range(2):
                        for j in range(2):
                            t = t2 + j
                            nc.tensor.matmul(
                                ps[:, col * 2 * K:(col + 1) * 2 * K],
                                lhsT=h_sel[:, t, mh * 128:(mh + 1) * 128],
                                rhs=pt_all[:, t, :],
                                start=True, stop=True)
                            col += 1
                    for mh in range(2):
                        nc.any.tensor_copy(
                            topkT[:, mh, t2 * 2 * K:(t2 + 2) * 2 * K],
                            ps[:, mh * 4 * K:(mh + 1) * 4 * K])

                # conv1d: y[p, oh, b, l] = relu(sum_{k, ih} w2^T topkT[:, b*30+l+k] + cb)
                y_sb = tp.tile([128, 2, GPC, L28], F32, tag="y")
                for oh in range(2):
                    for bc in range(GPC // GCHUNK):
                        ps = ptl.tile([128, GCHUNK, L28], F32, tag="psconv")
                        step = 0
                        for k in range(KS):
                            for ih in range(2):
                                base = topkT[:, ih, :]
                                rhs = _ap(base, bc * GCHUNK * K + k,
                                          [[K, GCHUNK], [1, L28]])
                                nc.tensor.matmul(
                                    ps[...],
                                    lhsT=w2[:, k * 2 + ih, oh * 128:(oh + 1) * 128],
                                    rhs=rhs,
                                    start=(step == 0), stop=(step == 2 * KS - 1))
                                step += 1
                        nc.scalar.activation(
                            y_sb[:, oh, bc * GCHUNK:(bc + 1) * GCHUNK, :], ps[...],
                            mybir.ActivationFunctionType.Relu,
                            bias=cb[:, oh:oh + 1])

                # lin1 (b-major): z1T[b, o] = relu(sum_s y_s^T @ w1_s + b1)
                # w1 slabs were prefetched into hT_b (0..31) and h_sb (32..53)
                ps1 = pfin.tile([GPC, H], F32, tag="ps1")
                for s in range(S1):
                    ot, l = divmod(s, L_OUT)
                    if s < W1A:
                        rhs = w1a[:, s, :]
                    else:
                        rhs = h_sb[:, s - W1A, :]
                    nc.tensor.matmul(
                        ps1[...],
                        lhsT=y_sb[:, ot, :, l],
                        rhs=rhs,
                        start=(s == 0), stop=(s == S1 - 1))
                z1t = tp.tile([GPC, H], F32, tag="z1t")
                nc.vector.tensor_add(z1t[...], ps1[...], b1[...])
                nc.scalar.activation(z1t[...], z1t[...],
                                     mybir.ActivationFunctionType.Relu, bias=0.0)
                # transpose z1T -> z1 [o on partitions]; all small psum tiles
                # below share one reused bank (sequential ops)
                z1 = tp.tile([128, 2, GPC], F32, tag="z1")
                for mh in range(2):
                    psz = pfin.tile([128, GPC], F32, tag="pfs")
                    nc.tensor.transpose(psz[...],
                                        z1t[:, mh * 128:(mh + 1) * 128],
                                        id128[0:GPC, 0:GPC])
                    nc.any.tensor_copy(z1[:, mh, :], psz[...])

                # lin2 + out
                ps2 = pfin.tile([128, GPC], F32, tag="pfs")
                for kh in range(2):
                    nc.tensor.matmul(ps2[...], lhsT=w4[:, kh, :], rhs=z1[:, kh, :],
                                     start=(kh == 0), stop=(kh == 1))
                z2 = tp.tile([128, GPC], F32, tag="z2")
                nc.scalar.activation(z2[...], ps2[...],
                                     mybir.ActivationFunctionType.Relu,
                                     bias=b2q[:, 0:1])
                ps3 = pfin.tile([128, GPC], F32, tag="pfs")
                nc.tensor.matmul(ps3[0:N_CLASSES, :], lhsT=w5[...], rhs=z2[...],
                                 start=True, stop=True)
                o_sb = tp.tile([N_CLASSES, GPC], F32, tag="osb")
                nc.scalar.activation(o_sb[...], ps3[0:N_CLASSES, :],
                                     mybir.ActivationFunctionType.Relu,
                                     bias=b3q[:, 0:1])
                nc.sync.dma_start(d["d_out"].ap().rearrange("b o -> o b"), o_sb[...])


# ---------------------------------------------------------------- entry point

_CACHED = {}


def _get_nc():
    if "nc" not in _CACHED:
        nc = bacc.Bacc("TRN2", target_bir_lowering=False, debug=False,
                       enable_asserts=True)
        _CACHED["nc"] = _build(nc)
    return _CACHED["nc"]


def make_in_maps(inputs):
    sh = _prep_shared(inputs)
    cores = _prep_cores(inputs)
    return [{**sh, **c} for c in cores]


TRACE = False


def kernel(**inputs):
    in_maps = make_in_maps(inputs)
    nc = _get_nc()
    res = run_bass_kernel_spmd(nc, in_maps, core_ids=list(range(N_CORES)),
                               trace=TRACE)
    _CACHED["last_res"] = res
    return np.concatenate([r["out"] for r in res.results], axis=0)


if __name__ == "__main__":
    import reference
    inputs = {k: np.asarray(v) for k, v in reference.setup_inputs().items()}
    out = kernel(**inputs)
    print("out", out.shape, out.dtype)


# revision 14
# speedup vs baseline: 1.1830x; 1.0474x over previous
"""DGCNN (4x SAGEConv + SortPool + Conv1d + MLP) Trainium2 Bass kernel.

Sharding: data-parallel over the B=512 graphs -> 64 graphs per core on 8 cores.
Edges never cross graphs, so each core's message passing is local. The edge
list is converted on the host into a per-graph normalized adjacency
(AT[g][s,d] = multiplicity(s->d) / max(deg(d),1)); aggregation then becomes a
block-diagonal dense matmul on the PE array (2 graphs of 64 nodes per
128-partition tile).

All pre-sort math is exact fp32: the sort keys have adjacent gaps down to
~3e-7, so any lower-precision SAGE arithmetic flips ranks and destroys the
output. Post-sort values tolerate ~1e-3 noise, so the selection matmul runs
in fp16 and conv/lin1 in f32r.

SortPool is computed exactly (stable argsort semantics incl. ties, which are
common: ~59% of keys are exactly 0 post-relu) via a rank computation:
  rank(i) = #{j : k_j > k_i}  on keys perturbed by  k_i -= i*1e-11
(the perturbation resolves exact ties by index; distinct key values are never
closer than ~3e-7 on this data so the perturbation cannot reorder them).
Selection of the top-30 rows per graph is a one-hot matmul.

Conv1d is 4 accumulated [128,*]x[128,*] matmuls per output tile (im2col via
strided access patterns, never materialized). lin1's 7MB weight is prefetched
at tail start into SBUF freed by the SAGE layers.
"""

import numpy as np

import concourse.bass as bass
import concourse.bacc as bacc
import concourse.mybir as mybir
import concourse.tile as tile
from concourse.bass_utils import run_bass_kernel_spmd

B, P, K, KS = 512, 64, 30, 4
N, E, F, H = B * P, 524288, 128, 256
L_OUT = K - KS + 1          # 27
N_CLASSES = 10
N_CORES = 8
GPC = B // N_CORES          # 64 graphs / core
NPC = GPC * P               # 4096 nodes / core
PAIRS = GPC // 2            # 32 pair-tiles (2 graphs of 64 nodes = 128 partitions)
NCHUNK = 512                # free-dim chunk for weight matmuls
F32 = mybir.dt.float32
F32R = mybir.dt.float32r
F16 = mybir.dt.float16
EPS_TIE = 1e-11

NLAYERS = 4
DUMP = False
USE_F32R = True
GCHUNK = 16                 # graphs per conv psum tile (16*28 = 448 <= 512)
L28 = L_OUT + 1             # conv free dim padded even (f32r ISA: innermost count even)
TKPAD = GPC * K + 8         # topkT free size incl. zeroed overrun pad
S1 = 2 * L_OUT              # 54 lin1 contraction steps of 128
W1A = 32                    # lin1 slabs prefetched into a fresh tail tile
W1B = S1 - W1A              # lin1 slabs prefetched into h_sb scratch


# ---------------------------------------------------------------- host prep

def _sp16(a):
    """Split fp32 -> (hi, lo) fp16 pair with hi + lo ~= a to ~2^-22 rel."""
    a = np.asarray(a, np.float32)
    hi = a.astype(np.float16)
    lo = (a - hi.astype(np.float32)).astype(np.float16)
    return np.ascontiguousarray(hi), np.ascontiguousarray(lo)


def _prep_shared(inp):
    """Host-side weight/constant reshaping (identical for every core)."""
    sh = {}
    for li in range(4):
        for nm in ("wl", "wr"):
            hi, lo = _sp16(inp[f"sage{li}_{nm}"])
            sh[f"{nm}{li}h"] = hi
            sh[f"{nm}{li}l"] = lo
        sh[f"b{li}"] = np.ascontiguousarray(inp[f"sage{li}_b"], np.float32)
    w = np.asarray(inp["conv1d_w"], np.float32)            # [O=256, I=256, KS]
    w2 = np.empty((2 * KS, 128, H), np.float16)
    for k in range(KS):
        wt = w[:, :, k].T                                  # [I, O]
        for ih in range(2):
            w2[k * 2 + ih] = wt[ih * 128:(ih + 1) * 128]
    sh["w2"] = w2
    sh["cb"] = np.ascontiguousarray(inp["conv1d_b"], np.float32)
    w1 = np.asarray(inp["lin1_w"], np.float32)             # [6912, 256]
    sh["w1"] = np.ascontiguousarray(
        w1.reshape(2, 128, L_OUT, H).transpose(0, 2, 1, 3).reshape(S1, 128, H)
        .astype(np.float16))
    sh["lb1"] = np.ascontiguousarray(
        np.broadcast_to(np.asarray(inp["lin1_b"], np.float32), (GPC, H)))
    sh["w4"] = np.ascontiguousarray(inp["lin2_w"], np.float32)   # [256, 128]
    sh["b2q"] = np.ascontiguousarray(inp["lin2_b"], np.float32)  # [128]
    sh["w5"] = np.ascontiguousarray(inp["out_w"], np.float32)    # [128, 10]
    sh["b3q"] = np.asarray(inp["out_b"], np.float32).reshape(N_CLASSES, 1).copy()
    sh["iota60"] = np.ascontiguousarray(
        np.broadcast_to(np.arange(2 * K, dtype=np.float32), (128, 2 * K)))
    off30 = np.zeros((128, 1), np.float32)
    off30[64:] = float(K)
    sh["off30"] = off30
    sh["epsrow"] = np.ascontiguousarray(
        np.broadcast_to(np.arange(P, dtype=np.float32) * np.float32(EPS_TIE), (P, P))).astype(np.float32)
    sh["id128"] = np.eye(128, dtype=np.float32)
    return sh


def _prep_cores(inp):
    """Per-core shards: node features (plain + transposed) and blockdiag adjacency."""
    x = np.nan_to_num(np.asarray(inp["x"], np.float32))
    ei = np.asarray(inp["edge_index"])
    src = ei[0].astype(np.int64)
    dst = ei[1].astype(np.int64)
    deg = np.bincount(dst, minlength=N).astype(np.float32)
    inv_deg = (1.0 / np.maximum(deg, 1.0)).astype(np.float32)
    g = src // P
    flat = g * (P * P) + (src % P) * P + (dst % P)
    AT = np.bincount(flat, minlength=B * P * P).astype(np.float32).reshape(B, P, P)
    AT *= inv_deg.reshape(B, P)[:, None, :]

    cores = []
    for c in range(N_CORES):
        xc = np.ascontiguousarray(x[c * NPC:(c + 1) * NPC])          # [4096, 128]
        atbd = np.zeros((PAIRS, 128, 128), np.float32)
        for t in range(PAIRS):
            atbd[t, :P, :P] = AT[c * GPC + 2 * t]
            atbd[t, P:, P:] = AT[c * GPC + 2 * t + 1]
        xth, xtl = _sp16(xc.T)                                   # [128, 4096]
        cores.append({
            "x": xc,
            "xth": xth,
            "xtl": xtl,
            "atbd": atbd,
        })
    return cores


# ---------------------------------------------------------------- device kernel

def _build(nc):
    """Emit the whole per-core kernel under a TileContext."""
    dt = nc.dram_tensor
    d_x = dt("x", [NPC, F], F32, kind="ExternalInput")
    WDT = F32R if USE_F32R else F32
    d_xth = dt("xth", [F, NPC], F16, kind="ExternalInput")
    d_xtl = dt("xtl", [F, NPC], F16, kind="ExternalInput")
    d_atbd = dt("atbd", [PAIRS, 128, 128], F32, kind="ExternalInput")
    d_wlh, d_wll, d_wrh, d_wrl, d_b = [], [], [], [], []
    for li in range(4):
        fin = F if li == 0 else H
        d_wlh.append(dt(f"wl{li}h", [fin, H], F16, kind="ExternalInput"))
        d_wll.append(dt(f"wl{li}l", [fin, H], F16, kind="ExternalInput"))
        d_wrh.append(dt(f"wr{li}h", [fin, H], F16, kind="ExternalInput"))
        d_wrl.append(dt(f"wr{li}l", [fin, H], F16, kind="ExternalInput"))
        d_b.append(dt(f"b{li}", [H], F32, kind="ExternalInput"))
    d_w2 = dt("w2", [2 * KS, 128, H], F16, kind="ExternalInput")
    d_cb = dt("cb", [H], F32, kind="ExternalInput")
    d_w1 = dt("w1", [S1, 128, H], F16, kind="ExternalInput")
    d_lb1 = dt("lb1", [GPC, H], F32, kind="ExternalInput")
    d_w4 = dt("w4", [H, 128], F32, kind="ExternalInput")
    d_b2q = dt("b2q", [128], F32, kind="ExternalInput")
    d_w5 = dt("w5", [128, N_CLASSES], F32, kind="ExternalInput")
    d_b3q = dt("b3q", [N_CLASSES, 1], F32, kind="ExternalInput")
    d_iota60 = dt("iota60", [128, 2 * K], F32, kind="ExternalInput")
    d_off30 = dt("off30", [128, 1], F32, kind="ExternalInput")
    d_epsrow = dt("epsrow", [P, P], F32, kind="ExternalInput")
    d_id128 = dt("id128", [128, 128], F32, kind="ExternalInput")
    d_out = dt("out", [GPC, N_CLASSES], F32, kind="ExternalOutput")
    if DUMP:
        d_dbg_ht = dt("dbg_ht", [128, 2, NPC], F32, kind="ExternalOutput")

    with tile.TileContext(nc) as tc:
        _emit(tc, nc, locals())
    nc.compile()
    return nc


def _ap(base, extra_offset, free_dims):
    """Build a custom AP view: keep base's partition dim, replace free dims."""
    return bass.AP(base.tensor, base.offset + extra_offset,
                   [base.ap[0]] + list(free_dims))


def _emit(tc, nc, d):
    WDT = F32R if USE_F32R else F32
    from contextlib import ExitStack
    ctx = ExitStack()
    with ctx:
        persist = ctx.enter_context(tc.tile_pool(name="persist", bufs=1))
        act_pool = ctx.enter_context(tc.tile_pool(name="acts", bufs=1))
        qs = [nc.sync, nc.scalar, nc.gpsimd]

        # ---- persistent loads (weight DMAs deferred until after input DMAs)
        _deferred = []

        def load(name, shape, view=None, dram=None, dtype=F32):
            t = persist.tile(shape, dtype, tag=name)
            src = (dram if dram is not None else d[f"d_{name}"]).ap()
            if view is not None:
                src = src.rearrange(*view[0], **view[1])
            _deferred.append((t, src))
            return t

        wlh, wll, wrh, wrl, bias = [], [], [], [], []
        for li in range(4):
            ki = 1 if li == 0 else 2
            vw = (["(k p) o -> p k o"], {"p": 128})
            wlh.append(load(f"wl{li}h", [128, ki, H], vw, dram=d["d_wlh"][li], dtype=F16))
            wll.append(load(f"wl{li}l", [128, ki, H], vw, dram=d["d_wll"][li], dtype=F16))
            wrh.append(load(f"wr{li}h", [128, ki, H], vw, dram=d["d_wrh"][li], dtype=F16))
            wrl.append(load(f"wr{li}l", [128, ki, H], vw, dram=d["d_wrl"][li], dtype=F16))
            bias.append(load(f"b{li}", [128, 2], (["(h p) -> p h"], {"p": 128}),
                             dram=d["d_b"][li]))
            if li == 0:
                # needed by the first L0 transposes: load early
                id128 = load("id128", [128, 128])
        w2 = load("w2", [128, 2 * KS, H], (["k p o -> p k o"], {}), dtype=F16)
        cb = load("cb", [128, 2], (["(h p) -> p h"], {"p": 128}))
        b1 = load("lb1", [GPC, H])
        w4 = load("w4", [128, 2, 128], (["(k p) o -> p k o"], {"p": 128}))
        b2q = load("b2q", [128, 1])
        w5 = load("w5", [128, N_CLASSES])
        b3q = load("b3q", [N_CLASSES, 1])
        iota60 = load("iota60", [128, 2 * K])
        off30 = load("off30", [128, 1])
        epsrow = load("epsrow", [P, P])

        # ---- activations (bufs=1: coarse WAR serialization at layer bounds is fine)
        # x is loaded directly into h_sb[:, :, 0:128] (L0 agg reads it there
        # before the L0 transposes overwrite it, tracked by tile deps).
        h_sb = act_pool.tile([128, PAIRS, H], F32, tag="h")       # nodes on partitions
        # single fp32 hT: consumed only by this layer's transposes (strictly
        # before the next layer's relu rewrites it); the weight-matmul rhs is
        # the fp16 hi/lo split pair below.
        hT = act_pool.tile([128, 2, NPC], F32, tag="hT")

        # ---- input x and aggregate (freed after the SAGE layers)
        with tc.tile_pool(name="xin", bufs=1) as xin:
            aggTh = xin.tile([128, 2, NPC], F16, tag="aggTh")
            aggTl = xin.tile([128, 2, NPC], F16, tag="aggTl")
            xth_sb = xin.tile([128, 1, NPC], F16, tag="xth")
            xtl_sb = xin.tile([128, 1, NPC], F16, tag="xtl")
            # fp16 hi/lo split of the layer's hT (weight-matmul rhs). Single
            # buffered: layer li+1 overwrites column sl only after both of its
            # oh-chunks for sl consumed it (ncki-outer loop order below).
            hsph = xin.tile([128, 2, NPC], F16, tag="hsph")
            hspl = xin.tile([128, 2, NPC], F16, tag="hspl")
            atbd_parts = []
            for g in range(4):
                src = d["d_x"].ap().rearrange("(t p) f -> p t f", p=128)[:, g * 8:(g + 1) * 8, :]
                qs[g % 3].dma_start(h_sb[:, g * 8:(g + 1) * 8, 0:F], src)
                t_at = persist.tile([128, PAIRS // 4, 128], F32, tag=f"atbd{g}",
                                    name=f"atbd{g}")
                srca = d["d_atbd"].ap().rearrange("t p n -> p t n")[:, g * 8:(g + 1) * 8, :]
                qs[(g + 1) % 3].dma_start(t_at[...], srca)
                atbd_parts.append(t_at)
                qs[(g + 2) % 3].dma_start(
                    xth_sb[:, 0, g * 1024:(g + 1) * 1024],
                    d["d_xth"].ap()[:, g * 1024:(g + 1) * 1024])
                qs[g % 3].dma_start(
                    xtl_sb[:, 0, g * 1024:(g + 1) * 1024],
                    d["d_xtl"].ap()[:, g * 1024:(g + 1) * 1024])

            for _i, (_t, _src) in enumerate(_deferred):
                qs[_i % 3].dma_start(_t[...], _src)
            _deferred.clear()

            with tc.tile_pool(name="ps_sage", bufs=3, space="PSUM") as psa, \
                 tc.tile_pool(name="ps_w", bufs=2, space="PSUM") as psw, \
                 tc.tile_pool(name="ps_tr", bufs=2, space="PSUM") as pst:
                for li in range(NLAYERS):
                    ki = 1 if li == 0 else 2
                    hTo = hT

                    # aggT[(i), n'] = h_pair^T @ ATbd_pair  per pair; 4 matmuls
                    # batched into one [128,512] psum tile -> wide copies
                    if li == 0:
                        for t4 in range(0, PAIRS, 4):
                            ps = psa.tile([128, 4 * 128], F32, tag="psa")
                            for j in range(4):
                                t = t4 + j
                                nc.tensor.matmul(
                                    ps[:, j * 128:(j + 1) * 128],
                                    lhsT=h_sb[:, t, 0:F],
                                    rhs=atbd_parts[t // 8][:, t % 8, :],
                                    start=True, stop=True)
                            dh = aggTh[:, 0, t4 * 128:(t4 + 4) * 128]
                            nc.any.tensor_copy(dh, ps[...])
                            nc.any.tensor_tensor(
                                aggTl[:, 0, t4 * 128:(t4 + 4) * 128],
                                ps[...], dh, op=mybir.AluOpType.subtract)
                    else:
                        for t2 in range(0, PAIRS, 2):
                            ps = psa.tile([128, 4 * 128], F32, tag="psa")
                            col = 0
                            for mh in range(2):
                                for j in range(2):
                                    t = t2 + j
                                    nc.tensor.matmul(
                                        ps[:, col * 128:(col + 1) * 128],
                                        lhsT=h_sb[:, t, mh * 128:(mh + 1) * 128],
                                        rhs=atbd_parts[t // 8][:, t % 8, :],
                                        start=True, stop=True)
                                    col += 1
                            for mh in range(2):
                                dh = aggTh[:, mh, t2 * 128:(t2 + 2) * 128]
                                nc.any.tensor_copy(dh, ps[:, mh * 256:(mh + 1) * 256])
                                nc.any.tensor_tensor(
                                    aggTl[:, mh, t2 * 128:(t2 + 2) * 128],
                                    ps[:, mh * 256:(mh + 1) * 256], dh,
                                    op=mybir.AluOpType.subtract)

                    # hT_next[o, n] = relu( wl^T aggT + wr^T hT + b )
                    # L3 computes oh=1 (the sort-key half) first so the sort can
                    # start while oh=0 is still on the PE.
                    if li == 0:
                        hh, hl = xth_sb, xtl_sb
                    else:
                        hh, hl = hsph, hspl

                    def _wchunk(oh, ncki):
                        sl = slice(ncki * NCHUNK, (ncki + 1) * NCHUNK)
                        ps = psw.tile([128, NCHUNK], F32, tag="psw", name="psw")
                        step, nsteps = 0, 2 * 3 * ki
                        for whi, wlo_, rth, rtl in ((wlh[li], wll[li], aggTh, aggTl),
                                                    (wrh[li], wrl[li], hh, hl)):
                            for kh in range(ki):
                                for wm, rt in ((whi, rth), (whi, rtl), (wlo_, rth)):
                                    nc.tensor.matmul(
                                        ps[...],
                                        lhsT=wm[:, kh, oh * 128:(oh + 1) * 128],
                                        rhs=rt[:, kh, sl],
                                        start=(step == 0), stop=(step == nsteps - 1))
                                    step += 1
                        if oh == 0:
                            nc.scalar.activation(
                                hTo[:, oh, sl], ps[...],
                                mybir.ActivationFunctionType.Relu,
                                bias=bias[li][:, oh:oh + 1])
                        else:
                            # relu on the vector engine: (ps + b) max 0
                            nc.vector.tensor_scalar(
                                hTo[:, oh, sl], ps[...],
                                bias[li][:, oh:oh + 1], 0.0,
                                op0=mybir.AluOpType.add,
                                op1=mybir.AluOpType.max)

                    if li == NLAYERS - 1:
                        # keys (oh=1 plane) first so the sort starts early
                        for oh in (1, 0):
                            for ncki in range(NPC // NCHUNK):
                                _wchunk(oh, ncki)
                    else:
                        for ncki in range(NPC // NCHUNK):
                            for oh in (0, 1):
                                _wchunk(oh, ncki)
                            # split columns sl for the next layer's fp16x2
                            # matmuls: hi = fp16(h), lo = fp16(h - hi)
                            sl = slice(ncki * NCHUNK, (ncki + 1) * NCHUNK)
                            for oh in (0, 1):
                                nc.any.tensor_copy(hsph[:, oh, sl], hTo[:, oh, sl])
                                nc.any.tensor_tensor(
                                    hspl[:, oh, sl], hTo[:, oh, sl],
                                    hsph[:, oh, sl], op=mybir.AluOpType.subtract)

                    # h_next = transpose(hT_next) per pair (PE transpose mode);
                    # both halves batched into one [128,256] psum -> one copy.
                    # L3's transposes happen in the tail (into fp16 h_sel).
                    if li < NLAYERS - 1:
                        for t in range(PAIRS):
                            ps = pst.tile([128, H], F32, tag="pst")
                            for oh in range(2):
                                nc.tensor.transpose(
                                    ps[:, oh * 128:(oh + 1) * 128],
                                    hTo[:, oh, t * 128:(t + 1) * 128],
                                    id128[...])
                            nc.any.tensor_copy(h_sb[:, t, :], ps[...])

        if DUMP:
            nc.sync.dma_start(d["d_dbg_ht"].ap(), hT[...])

        # ---------------- tail: sort + selection + conv + mlp.
        # hT_b and h_sb are dead after the L3 weight matmuls / aggs; their SBUF
        # is reused as the lin1 weight prefetch buffer (7MB streamed here,
        # hidden behind sort+selection+conv).
        with tc.tile_pool(name="sort", bufs=1) as sp, \
             tc.tile_pool(name="tail", bufs=1) as tp:
            w1qs = [nc.scalar, nc.gpsimd, nc.scalar, nc.gpsimd, nc.scalar, nc.gpsimd]
            w1f = tp.tile([128, S1, H], F16, tag="w1f")
            for i in range(6):          # 9 slabs per DMA
                s0 = 9 * i
                dst = w1f[:, s0:s0 + 9, :]
                src = d["d_w1"].ap()[s0:s0 + 9].rearrange("s p h -> p s h")
                w1qs[i].dma_start(dst, src)

            hTo = hT
            h_sel = tp.tile([128, PAIRS, H], F16, tag="hsel")
            with tc.tile_pool(name="ps_t3", bufs=2, space="PSUM") as pst3:
                for t in range(PAIRS):
                    ps = pst3.tile([128, H], F32, tag="pst3")
                    for oh in range(2):
                        nc.tensor.transpose(
                            ps[:, oh * 128:(oh + 1) * 128],
                            hTo[:, oh, t * 128:(t + 1) * 128],
                            id128[...])
                    nc.any.tensor_copy(h_sel[:, t, :], ps[...])

            # ---------------- sort: ranks of the last feature channel per graph
            rt = sp.tile([P, P], F32, tag="rt")
            with tc.tile_pool(name="sort_scratch", bufs=1) as ss:
                km = ss.tile([P, P], F32, tag="km")
                # keys: feature 255 = (hi=1, p=127); node n = g*64+i
                nc.sync.dma_start(km[...], hTo[127:128, 1, :])
                kmp = ss.tile([P, P], F32, tag="kmp")
                nc.vector.tensor_sub(kmp[...], km[...], epsrow[...])
                cbt = ss.tile([P, P * P], F32, tag="cbt")
                kb = kmp[:, :]
                in0 = _ap(kb, 0, [[0, P], kb.ap[1]])       # [g, i(bc), j]   k(g, j)
                in1 = _ap(kb, 0, [kb.ap[1], [0, P]])       # [g, i, j(bc)]   k(g, i)
                nc.vector.tensor_tensor(
                    _ap(cbt[:, :], 0, [[P, P], [1, P]]), in0, in1,
                    op=mybir.AluOpType.is_gt)
                rk = ss.tile([P, P], F32, tag="rk")
                nc.vector.tensor_reduce(
                    rk[...], _ap(cbt[:, :], 0, [[P, P], [1, P]]),
                    axis=mybir.AxisListType.X, op=mybir.AluOpType.add)
                # transpose ranks -> [node i, graph g]
                with tc.tile_pool(name="ps_sort", bufs=1, space="PSUM") as pss:
                    pr = pss.tile([P, P], F32, tag="pr")
                    nc.tensor.transpose(pr[...], rk[...], id128[0:P, 0:P])
                    nc.any.tensor_copy(rt[...], pr[...])
            # rankP[p, t] = rank(node p%64 of graph 2t + p//64)
            rankp = sp.tile([128, PAIRS], F32, tag="rankp")
            rb = rt[:, :]
            nc.vector.tensor_copy(rankp[0:P, :], _ap(rb, 0, [[2, PAIRS]]))
            nc.sync.dma_start(rankp[P:128, :], _ap(rb, 1, [[2, PAIRS]]))
            # rank2 = rankp + 30*(p>=64) + 1000*(rankp>=30)
            ge30 = sp.tile([128, PAIRS], F32, tag="ge30")
            nc.vector.tensor_scalar(ge30[...], rankp[...], float(K), None,
                                    op0=mybir.AluOpType.is_ge)
            rank2 = sp.tile([128, PAIRS], F32, tag="rank2")
            nc.vector.scalar_tensor_tensor(rank2[...], ge30[...], 1000.0,
                                           rankp[...], op0=mybir.AluOpType.mult,
                                           op1=mybir.AluOpType.add)
            nc.vector.tensor_scalar(rank2[...], rank2[...], off30[:, 0:1], None,
                                    op0=mybir.AluOpType.add)
            # one-hot selection matrices  PT[p, t, c] = (c == rank2[p, t])
            pt_all = sp.tile([128, PAIRS, 2 * K], F16, tag="pt")
            io = iota60[:, :]
            r2 = rank2[:, :]
            nc.vector.tensor_tensor(
                pt_all[...],
                _ap(io, 0, [[0, PAIRS], [1, 2 * K]]),
                _ap(r2, 0, [[1, PAIRS], [0, 2 * K]]),
                op=mybir.AluOpType.is_equal)

            # ---------------- selection + conv + mlp
            with tc.tile_pool(name="ps_tail", bufs=2, space="PSUM") as ptl, \
                 tc.tile_pool(name="ps_fin", bufs=1, space="PSUM") as pfin:
                # topkT[f, b*30+r] = sum_n h4[n, f] * PT[n, b(pair), r]
                # fp16 inputs (values only; ranks already decided) 2 pairs/psum
                topkT = tp.tile([128, 2, TKPAD], F16, tag="topkT")
                nc.vector.memset(topkT[:, :, GPC * K:].bitcast(F32), 0.0)
                for t2 in range(0, PAIRS, 2):
                    ps = ptl.tile([128, 8 * K], F32, tag="pssel")
                    col = 0
                    for mh in range(2):
                        for j in range(2):
                            t = t2 + j
                            nc.tensor.matmul(
                                ps[:, col * 2 * K:(col + 1) * 2 * K],
                                lhsT=h_sel[:, t, mh * 128:(mh + 1) * 128],
                                rhs=pt_all[:, t, :],
                                start=True, stop=True)
                            col += 1
                    for mh in range(2):
                        nc.any.tensor_copy(
                            topkT[:, mh, t2 * 2 * K:(t2 + 2) * 2 * K],
                            ps[:, mh * 4 * K:(mh + 1) * 4 * K])

                # conv1d: y[p, oh, b, l] = relu(sum_{k, ih} w2^T topkT[:, b*30+l+k] + cb)
                y_sb = tp.tile([128, 2, GPC, L28], F16, tag="y")
                for oh in range(2):
                    for bc in range(GPC // GCHUNK):
                        ps = ptl.tile([128, GCHUNK, L28], F32, tag="psconv")
                        step = 0
                        for k in range(KS):
                            for ih in range(2):
                                base = topkT[:, ih, :]
                                rhs = _ap(base, bc * GCHUNK * K + k,
                                          [[K, GCHUNK], [1, L28]])
                                nc.tensor.matmul(
                                    ps[...],
                                    lhsT=w2[:, k * 2 + ih, oh * 128:(oh + 1) * 128],
                                    rhs=rhs,
                                    start=(step == 0), stop=(step == 2 * KS - 1))
                                step += 1
                        nc.scalar.activation(
                            y_sb[:, oh, bc * GCHUNK:(bc + 1) * GCHUNK, :], ps[...],
                            mybir.ActivationFunctionType.Relu,
                            bias=cb[:, oh:oh + 1])

                # lin1 (b-major): z1T[b, o] = relu(sum_s y_s^T @ w1_s + b1)
                ps1 = pfin.tile([GPC, H], F32, tag="ps1")
                for s in range(S1):
                    ot, l = divmod(s, L_OUT)
                    rhs = w1f[:, s, :]
                    nc.tensor.matmul(
                        ps1[...],
                        lhsT=y_sb[:, ot, :, l],
                        rhs=rhs,
                        start=(s == 0), stop=(s == S1 - 1))
                z1t = tp.tile([GPC, H], F32, tag="z1t")
                nc.vector.tensor_add(z1t[...], ps1[...], b1[...])
                nc.scalar.activation(z1t[...], z1t[...],
                                     mybir.ActivationFunctionType.Relu, bias=0.0)
                # transpose z1T -> z1 [o on partitions]; all small psum tiles
                # below share one reused bank (sequential ops)
                z1 = tp.tile([128, 2, GPC], F32, tag="z1")
                for mh in range(2):
                    psz = pfin.tile([128, GPC], F32, tag="pfs")
                    nc.tensor.transpose(psz[...],
                                        z1t[:, mh * 128:(mh + 1) * 128],
                                        id128[0:GPC, 0:GPC])
                    nc.any.tensor_copy(z1[:, mh, :], psz[...])

                # lin2 + out
                ps2 = pfin.tile([128, GPC], F32, tag="pfs")
                for kh in range(2):
                    nc.tensor.matmul(ps2[...], lhsT=w4[:, kh, :], rhs=z1[:, kh, :],
                                     start=(kh == 0), stop=(kh == 1))
                z2 = tp.tile([128, GPC], F32, tag="z2")
                nc.scalar.activation(z2[...], ps2[...],
                                     mybir.ActivationFunctionType.Relu,
                                     bias=b2q[:, 0:1])
                ps3 = pfin.tile([128, GPC], F32, tag="pfs")
                nc.tensor.matmul(ps3[0:N_CLASSES, :], lhsT=w5[...], rhs=z2[...],
                                 start=True, stop=True)
                o_sb = tp.tile([N_CLASSES, GPC], F32, tag="osb")
                nc.scalar.activation(o_sb[...], ps3[0:N_CLASSES, :],
                                     mybir.ActivationFunctionType.Relu,
                                     bias=b3q[:, 0:1])
                nc.sync.dma_start(d["d_out"].ap().rearrange("b o -> o b"), o_sb[...])


# ---------------------------------------------------------------- entry point

_CACHED = {}


def _get_nc():
    if "nc" not in _CACHED:
        nc = bacc.Bacc("TRN2", target_bir_lowering=False, debug=False,
                       enable_asserts=True)
        _CACHED["nc"] = _build(nc)
    return _CACHED["nc"]


def make_in_maps(inputs):
    sh = _prep_shared(inputs)
    cores = _prep_cores(inputs)
    return [{**sh, **c} for c in cores]


TRACE = False


def kernel(**inputs):
    in_maps = make_in_maps(inputs)
    nc = _get_nc()
    res = run_bass_kernel_spmd(nc, in_maps, core_ids=list(range(N_CORES)),
                               trace=TRACE)
    _CACHED["last_res"] = res
    return np.concatenate([r["out"] for r in res.results], axis=0)


if __name__ == "__main__":
    import reference
    inputs = {k: np.asarray(v) for k, v in reference.setup_inputs().items()}
    out = kernel(**inputs)
    print("out", out.shape, out.dtype)


# revision 15
# speedup vs baseline: 1.1889x; 1.0050x over previous
"""DGCNN (4x SAGEConv + SortPool + Conv1d + MLP) Trainium2 Bass kernel.

Sharding: data-parallel over the B=512 graphs -> 64 graphs per core on 8 cores.
Edges never cross graphs, so each core's message passing is local. The edge
list is converted on the host into a per-graph normalized adjacency
(AT[g][s,d] = multiplicity(s->d) / max(deg(d),1)); aggregation then becomes a
block-diagonal dense matmul on the PE array (2 graphs of 64 nodes per
128-partition tile).

All pre-sort math is exact fp32: the sort keys have adjacent gaps down to
~3e-7, so any lower-precision SAGE arithmetic flips ranks and destroys the
output. Post-sort values tolerate ~1e-3 noise, so the selection matmul runs
in fp16 and conv/lin1 in f32r.

SortPool is computed exactly (stable argsort semantics incl. ties, which are
common: ~59% of keys are exactly 0 post-relu) via a rank computation:
  rank(i) = #{j : k_j > k_i}  on keys perturbed by  k_i -= i*1e-11
(the perturbation resolves exact ties by index; distinct key values are never
closer than ~3e-7 on this data so the perturbation cannot reorder them).
Selection of the top-30 rows per graph is a one-hot matmul.

Conv1d is 4 accumulated [128,*]x[128,*] matmuls per output tile (im2col via
strided access patterns, never materialized). lin1's 7MB weight is prefetched
at tail start into SBUF freed by the SAGE layers.
"""

import numpy as np

import concourse.bass as bass
import concourse.bacc as bacc
import concourse.mybir as mybir
import concourse.tile as tile
from concourse.bass_utils import run_bass_kernel_spmd

B, P, K, KS = 512, 64, 30, 4
N, E, F, H = B * P, 524288, 128, 256
L_OUT = K - KS + 1          # 27
N_CLASSES = 10
N_CORES = 8
GPC = B // N_CORES          # 64 graphs / core
NPC = GPC * P               # 4096 nodes / core
PAIRS = GPC // 2            # 32 pair-tiles (2 graphs of 64 nodes = 128 partitions)
NCHUNK = 512                # free-dim chunk for weight matmuls
F32 = mybir.dt.float32
F32R = mybir.dt.float32r
F16 = mybir.dt.float16
EPS_TIE = 1e-11

NLAYERS = 4
DUMP = False
USE_F32R = True
GCHUNK = 16                 # graphs per conv psum tile (16*28 = 448 <= 512)
L28 = L_OUT + 1             # conv free dim padded even (f32r ISA: innermost count even)
TKPAD = GPC * K + 8         # topkT free size incl. zeroed overrun pad
S1 = 2 * L_OUT              # 54 lin1 contraction steps of 128
W1A = 32                    # lin1 slabs prefetched into a fresh tail tile
W1B = S1 - W1A              # lin1 slabs prefetched into h_sb scratch


# ---------------------------------------------------------------- host prep

def _sp16(a):
    """Split fp32 -> (hi, lo) fp16 pair with hi + lo ~= a to ~2^-22 rel."""
    a = np.asarray(a, np.float32)
    hi = a.astype(np.float16)
    lo = (a - hi.astype(np.float32)).astype(np.float16)
    return np.ascontiguousarray(hi), np.ascontiguousarray(lo)


def _prep_shared(inp):
    """Host-side weight/constant reshaping (identical for every core)."""
    sh = {}
    for li in range(4):
        for nm in ("wl", "wr"):
            hi, lo = _sp16(inp[f"sage{li}_{nm}"])
            sh[f"{nm}{li}h"] = hi
            sh[f"{nm}{li}l"] = lo
        sh[f"b{li}"] = np.ascontiguousarray(inp[f"sage{li}_b"], np.float32)
    w = np.asarray(inp["conv1d_w"], np.float32)            # [O=256, I=256, KS]
    w2 = np.empty((2 * KS, 128, H), np.float16)
    for k in range(KS):
        wt = w[:, :, k].T                                  # [I, O]
        for ih in range(2):
            w2[k * 2 + ih] = wt[ih * 128:(ih + 1) * 128]
    sh["w2"] = w2
    sh["cb"] = np.ascontiguousarray(inp["conv1d_b"], np.float32)
    w1 = np.asarray(inp["lin1_w"], np.float32)             # [6912, 256]
    sh["w1"] = np.ascontiguousarray(
        w1.reshape(2, 128, L_OUT, H).transpose(0, 2, 1, 3).reshape(S1, 128, H)
        .astype(np.float16))
    sh["lb1"] = np.ascontiguousarray(
        np.broadcast_to(np.asarray(inp["lin1_b"], np.float32), (GPC, H)))
    sh["w4"] = np.ascontiguousarray(inp["lin2_w"], np.float32)   # [256, 128]
    sh["b2q"] = np.ascontiguousarray(inp["lin2_b"], np.float32)  # [128]
    sh["w5"] = np.ascontiguousarray(inp["out_w"], np.float32)    # [128, 10]
    sh["b3q"] = np.asarray(inp["out_b"], np.float32).reshape(N_CLASSES, 1).copy()
    sh["iota60"] = np.ascontiguousarray(
        np.broadcast_to(np.arange(2 * K, dtype=np.float32), (128, 2 * K)))
    off30 = np.zeros((128, 1), np.float32)
    off30[64:] = float(K)
    sh["off30"] = off30
    sh["epsrow"] = np.ascontiguousarray(
        np.broadcast_to(np.arange(P, dtype=np.float32) * np.float32(EPS_TIE), (P, P))).astype(np.float32)
    sh["id128"] = np.eye(128, dtype=np.float32)
    return sh


def _prep_cores(inp):
    """Per-core shards: node features (plain + transposed) and blockdiag adjacency."""
    x = np.nan_to_num(np.asarray(inp["x"], np.float32))
    ei = np.asarray(inp["edge_index"])
    src = ei[0].astype(np.int64)
    dst = ei[1].astype(np.int64)
    deg = np.bincount(dst, minlength=N).astype(np.float32)
    inv_deg = (1.0 / np.maximum(deg, 1.0)).astype(np.float32)
    g = src // P
    flat = g * (P * P) + (src % P) * P + (dst % P)
    AT = np.bincount(flat, minlength=B * P * P).astype(np.float32).reshape(B, P, P)
    AT *= inv_deg.reshape(B, P)[:, None, :]

    cores = []
    for c in range(N_CORES):
        xc = np.ascontiguousarray(x[c * NPC:(c + 1) * NPC])          # [4096, 128]
        atbd = np.zeros((PAIRS, 128, 128), np.float32)
        for t in range(PAIRS):
            atbd[t, :P, :P] = AT[c * GPC + 2 * t]
            atbd[t, P:, P:] = AT[c * GPC + 2 * t + 1]
        xth, xtl = _sp16(xc.T)                                   # [128, 4096]
        cores.append({
            "x": xc,
            "xth": xth,
            "xtl": xtl,
            "atbd": atbd,
        })
    return cores


# ---------------------------------------------------------------- device kernel

def _build(nc):
    """Emit the whole per-core kernel under a TileContext."""
    dt = nc.dram_tensor
    d_x = dt("x", [NPC, F], F32, kind="ExternalInput")
    WDT = F32R if USE_F32R else F32
    d_xth = dt("xth", [F, NPC], F16, kind="ExternalInput")
    d_xtl = dt("xtl", [F, NPC], F16, kind="ExternalInput")
    d_atbd = dt("atbd", [PAIRS, 128, 128], F32, kind="ExternalInput")
    d_wlh, d_wll, d_wrh, d_wrl, d_b = [], [], [], [], []
    for li in range(4):
        fin = F if li == 0 else H
        d_wlh.append(dt(f"wl{li}h", [fin, H], F16, kind="ExternalInput"))
        d_wll.append(dt(f"wl{li}l", [fin, H], F16, kind="ExternalInput"))
        d_wrh.append(dt(f"wr{li}h", [fin, H], F16, kind="ExternalInput"))
        d_wrl.append(dt(f"wr{li}l", [fin, H], F16, kind="ExternalInput"))
        d_b.append(dt(f"b{li}", [H], F32, kind="ExternalInput"))
    d_w2 = dt("w2", [2 * KS, 128, H], F16, kind="ExternalInput")
    d_cb = dt("cb", [H], F32, kind="ExternalInput")
    d_w1 = dt("w1", [S1, 128, H], F16, kind="ExternalInput")
    d_lb1 = dt("lb1", [GPC, H], F32, kind="ExternalInput")
    d_w4 = dt("w4", [H, 128], F32, kind="ExternalInput")
    d_b2q = dt("b2q", [128], F32, kind="ExternalInput")
    d_w5 = dt("w5", [128, N_CLASSES], F32, kind="ExternalInput")
    d_b3q = dt("b3q", [N_CLASSES, 1], F32, kind="ExternalInput")
    d_iota60 = dt("iota60", [128, 2 * K], F32, kind="ExternalInput")
    d_off30 = dt("off30", [128, 1], F32, kind="ExternalInput")
    d_epsrow = dt("epsrow", [P, P], F32, kind="ExternalInput")
    d_id128 = dt("id128", [128, 128], F32, kind="ExternalInput")
    d_out = dt("out", [GPC, N_CLASSES], F32, kind="ExternalOutput")
    if DUMP:
        d_dbg_ht = dt("dbg_ht", [128, 2, NPC], F32, kind="ExternalOutput")

    with tile.TileContext(nc) as tc:
        _emit(tc, nc, locals())
    nc.compile()
    return nc


def _ap(base, extra_offset, free_dims):
    """Build a custom AP view: keep base's partition dim, replace free dims."""
    return bass.AP(base.tensor, base.offset + extra_offset,
                   [base.ap[0]] + list(free_dims))


def _emit(tc, nc, d):
    WDT = F32R if USE_F32R else F32
    from contextlib import ExitStack
    ctx = ExitStack()
    with ctx:
        persist = ctx.enter_context(tc.tile_pool(name="persist", bufs=1))
        act_pool = ctx.enter_context(tc.tile_pool(name="acts", bufs=1))
        qs = [nc.sync, nc.scalar, nc.gpsimd]

        # ---- persistent loads (weight DMAs deferred until after input DMAs)
        _deferred = []

        def load(name, shape, view=None, dram=None, dtype=F32):
            t = persist.tile(shape, dtype, tag=name)
            src = (dram if dram is not None else d[f"d_{name}"]).ap()
            if view is not None:
                src = src.rearrange(*view[0], **view[1])
            _deferred.append((t, src))
            return t

        wlh, wll, wrh, wrl, bias = [], [], [], [], []
        for li in range(4):
            ki = 1 if li == 0 else 2
            vw = (["(k p) o -> p k o"], {"p": 128})
            wlh.append(load(f"wl{li}h", [128, ki, H], vw, dram=d["d_wlh"][li], dtype=F16))
            wll.append(load(f"wl{li}l", [128, ki, H], vw, dram=d["d_wll"][li], dtype=F16))
            wrh.append(load(f"wr{li}h", [128, ki, H], vw, dram=d["d_wrh"][li], dtype=F16))
            wrl.append(load(f"wr{li}l", [128, ki, H], vw, dram=d["d_wrl"][li], dtype=F16))
            bias.append(load(f"b{li}", [128, 2], (["(h p) -> p h"], {"p": 128}),
                             dram=d["d_b"][li]))
            if li == 0:
                # needed by the first L0 transposes: load early
                id128 = load("id128", [128, 128])
        w2 = load("w2", [128, 2 * KS, H], (["k p o -> p k o"], {}), dtype=F16)
        cb = load("cb", [128, 2], (["(h p) -> p h"], {"p": 128}))
        b1 = load("lb1", [GPC, H])
        w4 = load("w4", [128, 2, 128], (["(k p) o -> p k o"], {"p": 128}))
        b2q = load("b2q", [128, 1])
        w5 = load("w5", [128, N_CLASSES])
        b3q = load("b3q", [N_CLASSES, 1])
        iota60 = load("iota60", [128, 2 * K])
        off30 = load("off30", [128, 1])
        epsrow = load("epsrow", [P, P])

        # ---- activations (bufs=1: coarse WAR serialization at layer bounds is fine)
        # x is loaded directly into h_sb[:, :, 0:128] (L0 agg reads it there
        # before the L0 transposes overwrite it, tracked by tile deps).
        h_sb = act_pool.tile([128, PAIRS, H], F32, tag="h")       # nodes on partitions
        # single fp32 hT: consumed only by this layer's transposes (strictly
        # before the next layer's relu rewrites it); the weight-matmul rhs is
        # the fp16 hi/lo split pair below.
        hT = act_pool.tile([128, 2, NPC], F32, tag="hT")

        # ---- input x and aggregate (freed after the SAGE layers)
        with tc.tile_pool(name="xin", bufs=1) as xin:
            aggTh = xin.tile([128, 2, NPC], F16, tag="aggTh")
            aggTl = xin.tile([128, 2, NPC], F16, tag="aggTl")
            xth_sb = xin.tile([128, 1, NPC], F16, tag="xth")
            xtl_sb = xin.tile([128, 1, NPC], F16, tag="xtl")
            # fp16 hi/lo split of the layer's hT (weight-matmul rhs). Single
            # buffered: layer li+1 overwrites column sl only after both of its
            # oh-chunks for sl consumed it (ncki-outer loop order below).
            hsph = xin.tile([128, 2, NPC], F16, tag="hsph")
            hspl = xin.tile([128, 2, NPC], F16, tag="hspl")
            atbd_parts = []
            for g in range(4):
                src = d["d_x"].ap().rearrange("(t p) f -> p t f", p=128)[:, g * 8:(g + 1) * 8, :]
                qs[g % 3].dma_start(h_sb[:, g * 8:(g + 1) * 8, 0:F], src)
                t_at = persist.tile([128, PAIRS // 4, 128], F32, tag=f"atbd{g}",
                                    name=f"atbd{g}")
                srca = d["d_atbd"].ap().rearrange("t p n -> p t n")[:, g * 8:(g + 1) * 8, :]
                qs[(g + 1) % 3].dma_start(t_at[...], srca)
                atbd_parts.append(t_at)
                qs[(g + 2) % 3].dma_start(
                    xth_sb[:, 0, g * 1024:(g + 1) * 1024],
                    d["d_xth"].ap()[:, g * 1024:(g + 1) * 1024])
                qs[g % 3].dma_start(
                    xtl_sb[:, 0, g * 1024:(g + 1) * 1024],
                    d["d_xtl"].ap()[:, g * 1024:(g + 1) * 1024])

            for _i, (_t, _src) in enumerate(_deferred):
                qs[_i % 3].dma_start(_t[...], _src)
            _deferred.clear()

            with tc.tile_pool(name="ps_sage", bufs=4, space="PSUM") as psa, \
                 tc.tile_pool(name="ps_w", bufs=2, space="PSUM") as psw, \
                 tc.tile_pool(name="ps_tr", bufs=2, space="PSUM") as pst:
                for li in range(NLAYERS):
                    ki = 1 if li == 0 else 2
                    hTo = hT

                    # aggT[(i), n'] = h_pair^T @ ATbd_pair  per pair; 4 matmuls
                    # batched into one [128,512] psum tile -> wide copies
                    if li == 0:
                        for t4 in range(0, PAIRS, 4):
                            ps = psa.tile([128, 4 * 128], F32, tag="psa")
                            for j in range(4):
                                t = t4 + j
                                nc.tensor.matmul(
                                    ps[:, j * 128:(j + 1) * 128],
                                    lhsT=h_sb[:, t, 0:F],
                                    rhs=atbd_parts[t // 8][:, t % 8, :],
                                    start=True, stop=True)
                            dh = aggTh[:, 0, t4 * 128:(t4 + 4) * 128]
                            nc.any.tensor_copy(dh, ps[...])
                            nc.any.tensor_tensor(
                                aggTl[:, 0, t4 * 128:(t4 + 4) * 128],
                                ps[...], dh, op=mybir.AluOpType.subtract)
                    else:
                        for t2 in range(0, PAIRS, 2):
                            ps = psa.tile([128, 4 * 128], F32, tag="psa")
                            col = 0
                            for mh in range(2):
                                for j in range(2):
                                    t = t2 + j
                                    nc.tensor.matmul(
                                        ps[:, col * 128:(col + 1) * 128],
                                        lhsT=h_sb[:, t, mh * 128:(mh + 1) * 128],
                                        rhs=atbd_parts[t // 8][:, t % 8, :],
                                        start=True, stop=True)
                                    col += 1
                            for mh in range(2):
                                dh = aggTh[:, mh, t2 * 128:(t2 + 2) * 128]
                                nc.any.tensor_copy(dh, ps[:, mh * 256:(mh + 1) * 256])
                                nc.any.tensor_tensor(
                                    aggTl[:, mh, t2 * 128:(t2 + 2) * 128],
                                    ps[:, mh * 256:(mh + 1) * 256], dh,
                                    op=mybir.AluOpType.subtract)

                    # hT_next[o, n] = relu( wl^T aggT + wr^T hT + b )
                    # L3 computes oh=1 (the sort-key half) first so the sort can
                    # start while oh=0 is still on the PE.
                    if li == 0:
                        hh, hl = xth_sb, xtl_sb
                    else:
                        hh, hl = hsph, hspl

                    def _wchunk(oh, ncki):
                        sl = slice(ncki * NCHUNK, (ncki + 1) * NCHUNK)
                        ps = psw.tile([128, NCHUNK], F32, tag="psw", name="psw")
                        step, nsteps = 0, 2 * 3 * ki
                        for whi, wlo_, rth, rtl in ((wlh[li], wll[li], aggTh, aggTl),
                                                    (wrh[li], wrl[li], hh, hl)):
                            for kh in range(ki):
                                for wm, rt in ((whi, rth), (whi, rtl), (wlo_, rth)):
                                    nc.tensor.matmul(
                                        ps[...],
                                        lhsT=wm[:, kh, oh * 128:(oh + 1) * 128],
                                        rhs=rt[:, kh, sl],
                                        start=(step == 0), stop=(step == nsteps - 1))
                                    step += 1
                        if oh == 0:
                            nc.scalar.activation(
                                hTo[:, oh, sl], ps[...],
                                mybir.ActivationFunctionType.Relu,
                                bias=bias[li][:, oh:oh + 1])
                        else:
                            # relu on the vector engine: (ps + b) max 0
                            nc.vector.tensor_scalar(
                                hTo[:, oh, sl], ps[...],
                                bias[li][:, oh:oh + 1], 0.0,
                                op0=mybir.AluOpType.add,
                                op1=mybir.AluOpType.max)

                    if li == NLAYERS - 1:
                        # keys (oh=1 plane) first so the sort starts early
                        for oh in (1, 0):
                            for ncki in range(NPC // NCHUNK):
                                _wchunk(oh, ncki)
                    else:
                        for ncki in range(NPC // NCHUNK):
                            for oh in (0, 1):
                                _wchunk(oh, ncki)
                            # split columns sl for the next layer's fp16x2
                            # matmuls: hi = fp16(h), lo = fp16(h - hi)
                            sl = slice(ncki * NCHUNK, (ncki + 1) * NCHUNK)
                            for oh in (0, 1):
                                nc.any.tensor_copy(hsph[:, oh, sl], hTo[:, oh, sl])
                                nc.any.tensor_tensor(
                                    hspl[:, oh, sl], hTo[:, oh, sl],
                                    hsph[:, oh, sl], op=mybir.AluOpType.subtract)

                    # h_next = transpose(hT_next) per pair (PE transpose mode);
                    # both halves batched into one [128,256] psum -> one copy.
                    # L3's transposes happen in the tail (into fp16 h_sel).
                    if li < NLAYERS - 1:
                        for t in range(PAIRS):
                            ps = pst.tile([128, H], F32, tag="pst")
                            for oh in range(2):
                                nc.tensor.transpose(
                                    ps[:, oh * 128:(oh + 1) * 128],
                                    hTo[:, oh, t * 128:(t + 1) * 128],
                                    id128[...])
                            nc.any.tensor_copy(h_sb[:, t, :], ps[...])

        if DUMP:
            nc.sync.dma_start(d["d_dbg_ht"].ap(), hT[...])

        # ---------------- tail: sort + selection + conv + mlp.
        # hT_b and h_sb are dead after the L3 weight matmuls / aggs; their SBUF
        # is reused as the lin1 weight prefetch buffer (7MB streamed here,
        # hidden behind sort+selection+conv).
        with tc.tile_pool(name="sort", bufs=1) as sp, \
             tc.tile_pool(name="tail", bufs=1) as tp:
            w1qs = [nc.scalar, nc.gpsimd, nc.scalar, nc.gpsimd, nc.scalar, nc.gpsimd]
            w1f = tp.tile([128, S1, H], F16, tag="w1f")
            for i in range(6):          # 9 slabs per DMA
                s0 = 9 * i
                dst = w1f[:, s0:s0 + 9, :]
                src = d["d_w1"].ap()[s0:s0 + 9].rearrange("s p h -> p s h")
                w1qs[i].dma_start(dst, src)

            hTo = hT
            h_sel = tp.tile([128, PAIRS, H], F16, tag="hsel")
            with tc.tile_pool(name="ps_t3", bufs=2, space="PSUM") as pst3:
                for t in range(PAIRS):
                    ps = pst3.tile([128, H], F32, tag="pst3")
                    for oh in range(2):
                        nc.tensor.transpose(
                            ps[:, oh * 128:(oh + 1) * 128],
                            hTo[:, oh, t * 128:(t + 1) * 128],
                            id128[...])
                    nc.any.tensor_copy(h_sel[:, t, :], ps[...])

            # ---------------- sort: ranks of the last feature channel per graph
            rt = sp.tile([P, P], F32, tag="rt")
            with tc.tile_pool(name="sort_scratch", bufs=1) as ss:
                km = ss.tile([P, P], F32, tag="km")
                # keys: feature 255 = (hi=1, p=127); node n = g*64+i
                nc.sync.dma_start(km[...], hTo[127:128, 1, :])
                kmp = ss.tile([P, P], F32, tag="kmp")
                nc.vector.tensor_sub(kmp[...], km[...], epsrow[...])
                cbt = ss.tile([P, P * P], F32, tag="cbt")
                kb = kmp[:, :]
                in0 = _ap(kb, 0, [[0, P], kb.ap[1]])       # [g, i(bc), j]   k(g, j)
                in1 = _ap(kb, 0, [kb.ap[1], [0, P]])       # [g, i, j(bc)]   k(g, i)
                nc.vector.tensor_tensor(
                    _ap(cbt[:, :], 0, [[P, P], [1, P]]), in0, in1,
                    op=mybir.AluOpType.is_gt)
                rk = ss.tile([P, P], F32, tag="rk")
                nc.vector.tensor_reduce(
                    rk[...], _ap(cbt[:, :], 0, [[P, P], [1, P]]),
                    axis=mybir.AxisListType.X, op=mybir.AluOpType.add)
                # transpose ranks -> [node i, graph g]
                with tc.tile_pool(name="ps_sort", bufs=1, space="PSUM") as pss:
                    pr = pss.tile([P, P], F32, tag="pr")
                    nc.tensor.transpose(pr[...], rk[...], id128[0:P, 0:P])
                    nc.any.tensor_copy(rt[...], pr[...])
            # rankP[p, t] = rank(node p%64 of graph 2t + p//64)
            rankp = sp.tile([128, PAIRS], F32, tag="rankp")
            rb = rt[:, :]
            nc.vector.tensor_copy(rankp[0:P, :], _ap(rb, 0, [[2, PAIRS]]))
            nc.sync.dma_start(rankp[P:128, :], _ap(rb, 1, [[2, PAIRS]]))
            # rank2 = rankp + 30*(p>=64) + 1000*(rankp>=30)
            ge30 = sp.tile([128, PAIRS], F32, tag="ge30")
            nc.vector.tensor_scalar(ge30[...], rankp[...], float(K), None,
                                    op0=mybir.AluOpType.is_ge)
            rank2 = sp.tile([128, PAIRS], F32, tag="rank2")
            nc.vector.scalar_tensor_tensor(rank2[...], ge30[...], 1000.0,
                                           rankp[...], op0=mybir.AluOpType.mult,
                                           op1=mybir.AluOpType.add)
            nc.vector.tensor_scalar(rank2[...], rank2[...], off30[:, 0:1], None,
                                    op0=mybir.AluOpType.add)
            # one-hot selection matrices  PT[p, t, c] = (c == rank2[p, t])
            pt_all = sp.tile([128, PAIRS, 2 * K], F16, tag="pt")
            io = iota60[:, :]
            r2 = rank2[:, :]
            nc.vector.tensor_tensor(
                pt_all[...],
                _ap(io, 0, [[0, PAIRS], [1, 2 * K]]),
                _ap(r2, 0, [[1, PAIRS], [0, 2 * K]]),
                op=mybir.AluOpType.is_equal)

            # ---------------- selection + conv + mlp
            with tc.tile_pool(name="ps_tail", bufs=2, space="PSUM") as ptl, \
                 tc.tile_pool(name="ps_fin", bufs=1, space="PSUM") as pfin:
                # topkT[f, b*30+r] = sum_n h4[n, f] * PT[n, b(pair), r]
                # fp16 inputs (values only; ranks already decided) 2 pairs/psum
                topkT = tp.tile([128, 2, TKPAD], F16, tag="topkT")
                nc.vector.memset(topkT[:, :, GPC * K:].bitcast(F32), 0.0)
                for t2 in range(0, PAIRS, 2):
                    ps = ptl.tile([128, 8 * K], F32, tag="pssel")
                    col = 0
                    for mh in range(2):
                        for j in range(2):
                            t = t2 + j
                            nc.tensor.matmul(
                                ps[:, col * 2 * K:(col + 1) * 2 * K],
                                lhsT=h_sel[:, t, mh * 128:(mh + 1) * 128],
                                rhs=pt_all[:, t, :],
                                start=True, stop=True)
                            col += 1
                    for mh in range(2):
                        nc.any.tensor_copy(
                            topkT[:, mh, t2 * 2 * K:(t2 + 2) * 2 * K],
                            ps[:, mh * 4 * K:(mh + 1) * 4 * K])

                # conv1d: y[p, oh, b, l] = relu(sum_{k, ih} w2^T topkT[:, b*30+l+k] + cb)
                y_sb = tp.tile([128, 2, GPC, L28], F16, tag="y")
                for oh in range(2):
                    for bc in range(GPC // GCHUNK):
                        ps = ptl.tile([128, GCHUNK, L28], F32, tag="psconv")
                        step = 0
                        for k in range(KS):
                            for ih in range(2):
                                base = topkT[:, ih, :]
                                rhs = _ap(base, bc * GCHUNK * K + k,
                                          [[K, GCHUNK], [1, L28]])
                                nc.tensor.matmul(
                                    ps[...],
                                    lhsT=w2[:, k * 2 + ih, oh * 128:(oh + 1) * 128],
                                    rhs=rhs,
                                    start=(step == 0), stop=(step == 2 * KS - 1))
                                step += 1
                        nc.scalar.activation(
                            y_sb[:, oh, bc * GCHUNK:(bc + 1) * GCHUNK, :], ps[...],
                            mybir.ActivationFunctionType.Relu,
                            bias=cb[:, oh:oh + 1])

                # lin1 (b-major): z1T[b, o] = relu(sum_s y_s^T @ w1_s + b1)
                ps1 = pfin.tile([GPC, H], F32, tag="ps1")
                for s in range(S1):
                    ot, l = divmod(s, L_OUT)
                    rhs = w1f[:, s, :]
                    nc.tensor.matmul(
                        ps1[...],
                        lhsT=y_sb[:, ot, :, l],
                        rhs=rhs,
                        start=(s == 0), stop=(s == S1 - 1))
                z1t = tp.tile([GPC, H], F32, tag="z1t")
                nc.vector.tensor_add(z1t[...], ps1[...], b1[...])
                nc.scalar.activation(z1t[...], z1t[...],
                                     mybir.ActivationFunctionType.Relu, bias=0.0)
                # transpose z1T -> z1 [o on partitions]; all small psum tiles
                # below share one reused bank (sequential ops)
                z1 = tp.tile([128, 2, GPC], F32, tag="z1")
                for mh in range(2):
                    psz = pfin.tile([128, GPC], F32, tag="pfs")
                    nc.tensor.transpose(psz[...],
                                        z1t[:, mh * 128:(mh + 1) * 128],
                                        id128[0:GPC, 0:GPC])
                    nc.any.tensor_copy(z1[:, mh, :], psz[...])

                # lin2 + out
                ps2 = pfin.tile([128, GPC], F32, tag="pfs")
                for kh in range(2):
                    nc.tensor.matmul(ps2[...], lhsT=w4[:, kh, :], rhs=z1[:, kh, :],
                                     start=(kh == 0), stop=(kh == 1))
                z2 = tp.tile([128, GPC], F32, tag="z2")
                nc.scalar.activation(z2[...], ps2[...],
                                     mybir.ActivationFunctionType.Relu,
                                     bias=b2q[:, 0:1])
                ps3 = pfin.tile([128, GPC], F32, tag="pfs")
                nc.tensor.matmul(ps3[0:N_CLASSES, :], lhsT=w5[...], rhs=z2[...],
                                 start=True, stop=True)
                o_sb = tp.tile([N_CLASSES, GPC], F32, tag="osb")
                nc.scalar.activation(o_sb[...], ps3[0:N_CLASSES, :],
                                     mybir.ActivationFunctionType.Relu,
                                     bias=b3q[:, 0:1])
                nc.sync.dma_start(d["d_out"].ap().rearrange("b o -> o b"), o_sb[...])


# ---------------------------------------------------------------- entry point

_CACHED = {}


def _get_nc():
    if "nc" not in _CACHED:
        nc = bacc.Bacc("TRN2", target_bir_lowering=False, debug=False,
                       enable_asserts=True)
        _CACHED["nc"] = _build(nc)
    return _CACHED["nc"]


def make_in_maps(inputs):
    sh = _prep_shared(inputs)
    cores = _prep_cores(inputs)
    return [{**sh, **c} for c in cores]


TRACE = False


def kernel(**inputs):
    in_maps = make_in_maps(inputs)
    nc = _get_nc()
    res = run_bass_kernel_spmd(nc, in_maps, core_ids=list(range(N_CORES)),
                               trace=TRACE)
    _CACHED["last_res"] = res
    return np.concatenate([r["out"] for r in res.results], axis=0)


if __name__ == "__main__":
    import reference
    inputs = {k: np.asarray(v) for k, v in reference.setup_inputs().items()}
    out = kernel(**inputs)
    print("out", out.shape, out.dtype)
